# revision 1
# baseline (speedup 1.0000x reference)
"""DTAModel (drug-target affinity) Trainium2 kernel — 8-core SPMD, single launch.

Sharding: node shards of 4096 (GCN path, dst-sharded edge aggregation with
AllGather of h between layers) + pair shards of 64 (protein CNN + regressor).
All float compute on device; host only builds index/coefficient tables.

v2: L1 aggregates padded-x directly (no xw1 stage), host-precomputed
coefficient matrices (S) and pooling one-hots, protein groups interleaved into
the GCN tile loop, fp32r single-pass matmuls for the conv/regressor path.
"""
import os
import sys
import types

import numpy as np

N_NODES = 32768
N_EDGES = 131072
BATCH = 512
SEQ_LEN = 1000
VOCAB = 26
EMB = 128
NCORES = 8
NSH = N_NODES // NCORES      # 4096 nodes per core
BSH = BATCH // NCORES        # 64 pairs per core
NT = NSH // 128              # 32 node tiles per core

F32 = np.float32
LAST_RES = None


def _install_ntff_shim():
    if "antenv.axon_hooks" in sys.modules:
        return
    mod = types.ModuleType("antenv.axon_hooks")
    holder = {"h": None}
    mod.set_axon_ntff_profile_hook = lambda h: holder.__setitem__("h", h)
    mod.get_axon_ntff_profile_hook = lambda: holder["h"]
    sys.modules["antenv.axon_hooks"] = mod
    try:
        from trn_agent_boot.trn_boot import _ntff_profile_via_ctypes
        so = "/opt/axon/libaxon_pjrt.so"
        if os.path.exists(so):
            mod.set_axon_ntff_profile_hook(_ntff_profile_via_ctypes(so))
    except Exception:
        pass


def _prep_edges(edge_index):
    """Edge tables: per-core dst-sorted 128-edge blocks + dense S matrices."""
    src = np.asarray(edge_index[0], np.int64)
    dst = np.asarray(edge_index[1], np.int64)
    deg = (1.0 + np.bincount(dst, minlength=N_NODES)).astype(F32)
    dis = (1.0 / np.sqrt(deg)).astype(F32)
    allsrc = np.concatenate([src, np.arange(N_NODES, dtype=np.int64)])
    alldst = np.concatenate([dst, np.arange(N_NODES, dtype=np.int64)])
    allcoef = np.concatenate([dis[src] * dis[dst], dis * dis]).astype(F32)
    order = np.argsort(alldst, kind="stable")
    s_s, d_s, c_s = allsrc[order], alldst[order], allcoef[order]

    tile_of = d_s // 128
    counts = np.bincount(tile_of, minlength=N_NODES // 128)
    bpt = int(np.ceil(counts.max() / 128))
    nblk = NT * bpt

    idx = np.zeros((NCORES, nblk * 128), np.int64)
    dstl = np.zeros((NCORES, nblk * 128), np.int64)
    coef = np.zeros((NCORES, nblk * 128), F32)
    tstart = np.concatenate([[0], np.cumsum(counts)])
    for gt in range(N_NODES // 128):
        c, t = divmod(gt, NT)
        lo, hi = tstart[gt], tstart[gt + 1]
        n = hi - lo
        base = t * bpt * 128
        idx[c, base:base + n] = s_s[lo:hi]
        dstl[c, base:base + n] = d_s[lo:hi] - gt * 128
        coef[c, base:base + n] = c_s[lo:hi]

    # dense S: S_all[c][e, 128*b + j] = coef * (dstl == j)
    S_all = np.zeros((NCORES, nblk, 128, 128), F32)
    bix = np.tile(np.arange(nblk)[:, None], (1, 128)).ravel()
    eix = np.tile(np.arange(128)[None, :], (nblk, 1)).ravel()
    for c in range(NCORES):
        S_all[c][bix, eix, dstl[c]] = coef[c]
    S_all = S_all.transpose(0, 2, 1, 3).reshape(NCORES, 128, nblk * 128).copy()

    nchunk = (nblk * 128) // 2048
    idx16 = idx.astype(np.int16)
    wrapped = np.zeros((NCORES, 128, nchunk * 128), np.int16)
    for c in range(NCORES):
        w = idx16[c].reshape(nchunk, 128, 16)
        for ci in range(nchunk):
            blockw = w[ci].reshape(-1, 16).T
            wrapped[c, :, 128 * ci:128 * (ci + 1)] = np.tile(blockw, (8, 1))
    return bpt, nblk, nchunk, wrapped, S_all


def kernel(**inputs):
    global LAST_RES
    _install_ntff_shim()
    SKIP_PROT = bool(int(os.environ.get("DTA_SKIP_PROT", "0")))
    SKIP_GCN = bool(int(os.environ.get("DTA_SKIP_GCN", "0")))
    F32R = bool(int(os.environ.get("DTA_F32R", "1")))
    import concourse.bacc as bacc
    import concourse.tile as tile
    from concourse import hw_specs
    # Tile's static schedule uses this cost model; the default badly
    # underestimates SWDGE gather descriptor generation (~8 ns/desc measured),
    # which starves the PE stream of filler work during gather windows.
    hw_specs.TRN2Spec.SWDGE_NS_PER_DESCRIPTOR = 8.0
    from concourse import mybir
    from concourse.bass_utils import run_bass_kernel_spmd
    from concourse.library_config import mlp as mlp_lib

    g = lambda k: np.ascontiguousarray(np.asarray(inputs[k], F32))
    x = g("x")
    batch = np.asarray(inputs["batch"], np.int64)
    seq = np.asarray(inputs["protein_seq"], np.int64)
    W1 = g("W1")
    W2 = g("W2")
    W3 = g("W3")
    g1, bt1, g2, bt2, g3, bt3 = g("g1"), g("bt1"), g("g2"), g("bt2"), g("g3"), g("bt3")
    emb_w = g("emb")
    ck1, cb1, ck2, cb2, ck3, cb3 = g("ck1"), g("cb1"), g("ck2"), g("cb2"), g("ck3"), g("cb3")
    fw1, fb1, fw2, fb2, fw3, fb3 = g("fw1"), g("fb1"), g("fw2"), g("fb2"), g("fw3"), g("fb3")

    # ---------------- host tables ----------------
    bpt, nblk, nchunk, idx_w, S_host = _prep_edges(np.asarray(inputs["edge_index"]))

    cnt = np.bincount(batch, minlength=BATCH).astype(F32)
    cntinv = (1.0 / np.maximum(cnt, 1.0)).astype(F32)
    # pooling one-hots: P_all[c][n, (w*NT + t)*128 + j] = (batch[node] == 128w + j)
    P_host = np.zeros((NCORES, 128, 4 * NT * 128), F32)
    for c in range(NCORES):
        bl = batch[NSH * c:NSH * (c + 1)].reshape(NT, 128)
        for t in range(NT):
            w = bl[t] // 128
            j = bl[t] % 128
            P_host[c, np.arange(128), (w * NT + t) * 128 + j] = 1.0

    chvec = np.zeros((128, 6), F32)
    chvec[:64, 0], chvec[:64, 1] = g1, bt1
    chvec[:, 2], chvec[:, 3] = g2, bt2
    chvec[:, 4], chvec[:, 5] = g3, bt3

    pidx = np.arange(128, dtype=F32)[:, None]
    ident = np.eye(128, dtype=F32)
    xpad = np.zeros((N_NODES, 64), F32)
    xpad[:, :5] = x

    ids4 = seq.astype(F32).reshape(NCORES, BSH // 4, 4, SEQ_LEN).transpose(0, 2, 1, 3) \
        .reshape(NCORES, 4, (BSH // 4) * SEQ_LEN).copy()
    sel4h = np.zeros((4, 4 * VOCAB), F32)
    for s in range(4):
        sel4h[s, VOCAB * s:VOCAB * (s + 1)] = 1.0
    ck1T = np.concatenate([ck1[:, :, t].T for t in range(4)], axis=1)
    ck2q = np.concatenate([np.tile(ck2[:, :, t].T, (4, 1)) for t in range(6)], axis=1)
    ck3d = np.concatenate([np.tile(ck3[:, :, t].T, (2, 1)) for t in range(8)], axis=1)
    fw1a = np.ascontiguousarray(fw1[:128])
    fw1b = np.ascontiguousarray(fw1[128:224])
    fb1c = np.ascontiguousarray(fb1.reshape(4, 128).T)
    fw2p = np.ascontiguousarray(fw2.reshape(4, 128, 256).transpose(1, 0, 2).reshape(128, 1024))
    fb2c = np.ascontiguousarray(fb2.reshape(2, 128).T)
    fw3p = np.ascontiguousarray(fw3.reshape(2, 128).T)

    # ---------------- device program ----------------
    dt = mybir.dt
    AF = mybir.ActivationFunctionType
    OP = mybir.AluOpType
    rg = [list(range(NCORES))]

    fr = dt.float32r if F32R else dt.float32

    nc = bacc.Bacc("TRN2", target_bir_lowering=False, debug=False, num_devices=NCORES)

    def din(name, shape, dty=dt.float32):
        return nc.dram_tensor(name, list(shape), dty, kind="ExternalInput")

    xpad_d = din("xpad", [N_NODES, 64], fr)
    W1_d, W2_d, W3_d = din("W1", [5, 64], fr), din("W2", [64, 128], fr), din("W3", [128, 128], fr)
    chv_d = din("chv", [128, 6])
    ident_d = din("ident", [128, 128])
    pidx_d = din("pidx", [128, 1])
    idx_d = din("idxg", [128, nchunk * 128], dt.int16)
    S_d = din("Sm", [128, nblk * 128], fr)
    P_d = din("Pm", [128, 4 * NT * 128], fr)
    cntinv_d = din("cntinv", [64, 1])
    ids_d = din("ids4", [4, (BSH // 4) * SEQ_LEN], fr)
    emb_d = din("embw", [VOCAB, EMB], fr)
    sel_d = din("sel4", [4, 4 * VOCAB], fr)
    ck1_d, ck2_d, ck3_d = din("ck1T", [128, 128]), din("ck2q", [128, 384], fr), din("ck3d", [128, 768], fr)
    cb1_d, cb2_d, cb3_d = din("cb1", [32, 1]), din("cb2", [64, 1]), din("cb3", [96, 1])
    f1a_d, f1b_d = din("fw1a", [128, 512], fr), din("fw1b", [96, 512], fr)
    fb1_d = din("fb1c", [128, 4])
    f2_d, fb2_d = din("fw2p", [128, 1024], fr), din("fb2c", [128, 2])
    f3_d, fb3_d = din("fw3p", [128, 2], fr), din("fb3", [1, 1])
    out_d = nc.dram_tensor("out", [1, BSH], dt.float32, kind="ExternalOutput")

    with tile.TileContext(nc) as tc:
        with (
            tc.tile_pool(name="dram", bufs=1, space="DRAM") as dram,
            tc.tile_pool(name="cst", bufs=1) as cst,
            tc.tile_pool(name="big", bufs=1) as big,
            tc.tile_pool(name="wk", bufs=2) as wk,
            tc.tile_pool(name="wk1", bufs=1) as wk1,
            tc.tile_pool(name="wk3", bufs=3) as wk3,
            tc.tile_pool(name="gp", bufs=2, space="PSUM") as gp,
            tc.tile_pool(name="pp", bufs=4, space="PSUM") as pp,
        ):
            nc.gpsimd.load_library(mlp_lib)

            ag1_in = dram.tile([NSH, 64], fr)
            h1_full = dram.tile([N_NODES, 64], fr, addr_space="Shared")
            ag2_in = dram.tile([NSH, 128], fr)
            h2_full = dram.tile([N_NODES, 128], fr, addr_space="Shared")
            ar_ins = [dram.tile([128, 2], dt.float32, name=f"arin{i}") for i in range(3)]
            ar_outs = [dram.tile([128, 2], dt.float32, addr_space="Shared",
                                 name=f"arout{i}") for i in range(3)]
            arp_in = dram.tile([BATCH, 128], dt.float32)
            rsp_out = dram.tile([BSH, 128], dt.float32)

            def load(name, d, shape, dty=dt.float32):
                t = cst.tile(shape, dty, name=name)
                nc.sync.dma_start(t[:], d[:])
                return t

            ident_t = load("ident_t", ident_d, [128, 128])
            pidx_t = load("pidx_t", pidx_d, [128, 1])
            chv_t = load("chv_t", chv_d, [128, 6])
            W1_t = load("W1_t", W1_d, [5, 64], fr)
            W2_t = load("W2_t", W2_d, [64, 128], fr)
            W3_t = load("W3_t", W3_d, [128, 128], fr)
            idx_t = load("idx_t", idx_d, [128, nchunk * 128], dt.int16)
            cntinv_t = load("cntinv_t", cntinv_d, [64, 1])
            ids_t = load("ids_t", ids_d, [4, (BSH // 4) * SEQ_LEN], fr)
            emb_t = load("emb_t", emb_d, [VOCAB, EMB], fr)
            sel_t = load("sel_t", sel_d, [4, 4 * VOCAB], fr)
            ck1_t = load("ck1_t", ck1_d, [128, 128])
            ck2_t = load("ck2_t", ck2_d, [128, 384], fr)
            ck3_t = load("ck3_t", ck3_d, [128, 768], fr)
            cb1_t = load("cb1_t", cb1_d, [32, 1])
            cb2_t = load("cb2_t", cb2_d, [64, 1])
            cb3_t = load("cb3_t", cb3_d, [96, 1])
            f1a_t = load("f1a_t", f1a_d, [128, 512], fr)
            f1b_t = load("f1b_t", f1b_d, [96, 512], fr)
            fb1_t = load("fb1_t", fb1_d, [128, 4])
            f2_t = load("f2_t", f2_d, [128, 1024], fr)
            fb2_t = load("fb2_t", fb2_d, [128, 2])
            f3_t = load("f3_t", f3_d, [128, 2], fr)
            fb3_t = load("fb3_t", fb3_d, [1, 1])

            z_sb = big.tile([128, NSH], dt.float32)
            h3T = big.tile([128, NSH], dt.float32)
            hnat = big.tile([128, NSH], fr)
            protT = big.tile([96, BSH], fr)
            c1T = [big.tile([128, BSH], fr, name=f"c1T_{j}") for j in range(4)]
            c2T = [big.tile([128, BSH], fr, name=f"c2T_{j}") for j in range(2)]

            # ---------------- protein group ----------------
            def protein_group(gi):
                xq = []
                for si in range(4):
                    xq_s = wk1.tile([128, 1002], dt.float32, tag=f"xq{si}",
                                    name=f"xq{gi}_{si}")
                    nc.vector.memset(xq_s[:, 0:1], 0.0)
                    nc.vector.memset(xq_s[:, 1001:1002], 0.0)
                    xq.append(xq_s)
                for lc in range(2):
                    l0 = 500 * lc
                    for si in range(4):
                        pb = pp.tile([VOCAB, 500], dt.float32, space="PSUM", tag="pp",
                                     name=f"pb{gi}_{lc}_{si}")
                        nc.tensor.matmul(
                            pb[:], lhsT=sel_t[:, VOCAB * si:VOCAB * (si + 1)],
                            rhs=ids_t[:, SEQ_LEN * gi + l0:SEQ_LEN * gi + l0 + 500],
                            start=True, stop=True)
                        oh = wk.tile([VOCAB, 500], fr, tag="oh",
                                     name=f"oh{gi}_{lc}_{si}")
                        nc.vector.tensor_scalar(oh[:], pb[:], pidx_t[:VOCAB, :], None,
                                                OP.is_equal)
                        pe = pp.tile([128, 500], dt.float32, space="PSUM", tag="pp",
                                     name=f"pe{gi}_{lc}_{si}")
                        nc.tensor.matmul(pe[:], lhsT=emb_t[:], rhs=oh[:],
                                         start=True, stop=True)
                        nc.scalar.activation(xq[si][:, 1 + l0:1 + l0 + 500], pe[:], AF.Copy)
                h1Q = wk.tile([128, 1003], fr, tag="h1Q", name=f"h1Q{gi}")
                nc.vector.memset(h1Q[:, 0:2].bitcast(dt.float32), 0.0)
                nc.vector.memset(h1Q[:, 1001:1003].bitcast(dt.float32), 0.0)
                for (l0, Lc) in ((0, 512), (512, 487)):
                    c1p = pp.tile([128, 512], dt.float32, space="PSUM", tag="pp",
                                  name=f"c1p{gi}_{l0}")
                    for tap in range(4):
                        for si in range(4):
                            nc.tensor.matmul(
                                c1p[32 * si:32 * (si + 1), :Lc],
                                lhsT=ck1_t[:, 32 * tap:32 * (tap + 1)],
                                rhs=xq[si][:, l0 + tap:l0 + tap + Lc],
                                start=(tap == 0), stop=(tap == 3),
                                tile_position=(0, 32 * si))
                    for si in range(4):
                        nc.scalar.activation(
                            h1Q[32 * si:32 * (si + 1), 2 + l0:2 + l0 + Lc],
                            c1p[32 * si:32 * (si + 1), :Lc], AF.Relu, bias=cb1_t[:])
                h2D = []
                for p in range(2):
                    h2p = wk.tile([128, 1005], fr, tag=f"h2D{p}",
                                  name=f"h2D{gi}_{p}")
                    nc.vector.memset(h2p[:, 0:3].bitcast(dt.float32), 0.0)
                    nc.vector.memset(h2p[:, 1001:1005].bitcast(dt.float32), 0.0)
                    h2D.append(h2p)
                for (l0, Lc) in ((0, 512), (512, 486)):
                    c2ps = []
                    for si in range(4):
                        c2p = pp.tile([64, 512], dt.float32, space="PSUM", tag="pp",
                                      name=f"c2p{gi}_{l0}_{si}")
                        c2ps.append(c2p)
                    for tap in range(6):
                        for si in range(4):
                            nc.tensor.matmul(
                                c2ps[si][:, :Lc],
                                lhsT=ck2_t[32 * si:32 * (si + 1),
                                              64 * tap:64 * (tap + 1)],
                                rhs=h1Q[32 * si:32 * (si + 1), l0 + tap:l0 + tap + Lc],
                                start=(tap == 0), stop=(tap == 5),
                                tile_position=(32 * si, 0))
                    for si in range(4):
                        nc.scalar.activation(
                            h2D[si // 2][64 * (si % 2):64 * (si % 2 + 1),
                                         3 + l0:3 + l0 + Lc],
                            c2ps[si][:, :Lc], AF.Relu, bias=cb2_t[:])
                for p in range(2):
                    mx = [wk.tile([96, 1], dt.float32, tag=f"mx{j}",
                                  name=f"mx{gi}_{p}_{j}") for j in range(2)]
                    tmp = [wk.tile([96, 1], dt.float32, tag=f"tm{j}",
                                   name=f"tm{gi}_{p}_{j}") for j in range(2)]
                    for (l0, Lc) in ((0, 512), (512, 486)):
                        Lreal = 512 if l0 == 0 else 485
                        c3ps = []
                        for j in range(2):
                            c3p = pp.tile([96, 512], dt.float32, space="PSUM", tag="pp",
                                          name=f"c3p{gi}_{p}_{l0}_{j}")
                            c3ps.append(c3p)
                        for tap in range(8):
                            for j in range(2):
                                nc.tensor.matmul(
                                    c3ps[j][:, :Lc],
                                    lhsT=ck3_t[64 * j:64 * (j + 1),
                                                  96 * tap:96 * (tap + 1)],
                                    rhs=h2D[p][64 * j:64 * (j + 1),
                                                  l0 + tap:l0 + tap + Lc],
                                    start=(tap == 0), stop=(tap == 7),
                                    tile_position=(64 * j, 0))
                        for j in range(2):
                            dst = mx[j] if l0 == 0 else tmp[j]
                            nc.vector.tensor_reduce(dst[:], c3ps[j][:, :Lreal],
                                                    axis=mybir.AxisListType.X, op=OP.max)
                            if l0 != 0:
                                nc.vector.tensor_tensor(mx[j][:], mx[j][:], tmp[j][:],
                                                        OP.max)
                    for j in range(2):
                        s_idx = 4 * gi + 2 * p + j
                        nc.scalar.activation(protT[:, s_idx:s_idx + 1], mx[j][:],
                                             AF.Relu, bias=cb3_t[:])

            pending = [] if SKIP_PROT else list(range(16))
            slot = [0]

            def filler(period=8):
                slot[0] += 1
                if pending and slot[0] % period == 0:
                    protein_group(pending.pop(0))

            # ---------------- GCN layer ----------------
            gsem = nc.alloc_semaphore("gsem")

            def gcn_layer(L, fg, fin, fout, src_dram, Wt):
                """fg: gathered row width; fin: contraction width of Wt."""
                zs = cst.tile([128, NT], dt.float32, name=f"zs{L}")
                zq = cst.tile([128, NT], dt.float32, name=f"zq{L}")
                sq_scr = wk.tile([128, 128], dt.float32, tag="sqs", name=f"sqs{L}")
                Gt = None
                for t in range(NT):
                    St = wk.tile([128, bpt * 128], fr, tag="Sld",
                                  name=f"S{L}_{t}")
                    nc.sync.dma_start(St[:], S_d[:, t * bpt * 128:(t + 1) * bpt * 128])
                    aggT = gp.tile([128, 128], dt.float32, space="PSUM", tag="aggp",
                                   name=f"agg{L}_{t}")
                    for k in range(bpt):
                        b = t * bpt + k
                        ci, bb = divmod(b, 16)
                        if bb == 0:
                            Gt = wk.tile([128, 16, fg], fr, tag="gch",
                                         name=f"g{L}_{ci}")
                            nc.gpsimd.dma_gather(
                                Gt[:], src_dram[:],
                                idx_t[:, 128 * ci:128 * (ci + 1)], 2048, 2048, fg,
                                single_packet=False)
                        nc.tensor.matmul(aggT[:fin, :], lhsT=Gt[:, bb, :fin],
                                         rhs=St[:, 128 * k:128 * (k + 1)],
                                         start=(k == 0), stop=(k == bpt - 1))
                    aggS = wk.tile([fin, 128], fr, tag="aggS",
                                   name=f"aggS{L}_{t}")
                    nc.vector.tensor_copy(aggS[:], aggT[:fin, :])
                    zT = gp.tile([128, 128], dt.float32, space="PSUM", tag="zp",
                                 name=f"z{L}_{t}")
                    nc.tensor.matmul(zT[:fout, :], lhsT=Wt[:fin, :fout], rhs=aggS[:],
                                     start=True, stop=True)
                    nc.scalar.activation(z_sb[:fout, 128 * t:128 * (t + 1)], zT[:fout, :],
                                         AF.Copy, accum_out=zs[:fout, t:t + 1])
                    nc.scalar.activation(sq_scr[:fout, :], zT[:fout, :], AF.Square,
                                         accum_out=zq[:fout, t:t + 1])
                    filler()
                ssum = wk.tile([128, 2], dt.float32, tag="ssum", name=f"ssum{L}")
                nc.vector.memset(ssum[:], 0.0)
                nc.vector.tensor_reduce(ssum[:fout, 0:1], zs[:fout, :],
                                        axis=mybir.AxisListType.X, op=OP.add)
                nc.vector.tensor_reduce(ssum[:fout, 1:2], zq[:fout, :],
                                        axis=mybir.AxisListType.X, op=OP.add)
                nc.sync.dma_start(ar_ins[L][:], ssum[:])
                nc.gpsimd.collective_compute(
                    "AllReduce", OP.add, replica_groups=rg,
                    ins=[ar_ins[L].opt()], outs=[ar_outs[L].opt()])
                stg = wk.tile([128, 2], dt.float32, tag="stg", name=f"stg{L}")
                nc.sync.dma_start(stg[:], ar_outs[L][:])
                vg = chv_t[:fout, 2 * L:2 * L + 1]
                vbt = chv_t[:fout, 2 * L + 1:2 * L + 2]
                mean = wk.tile([128, 1], dt.float32, tag="bnv0", name=f"mean{L}")
                ex2 = wk.tile([128, 1], dt.float32, tag="bnv1", name=f"ex2{L}")
                var = wk.tile([128, 1], dt.float32, tag="bnv2", name=f"var{L}")
                sd = wk.tile([128, 1], dt.float32, tag="bnv3", name=f"sd{L}")
                s_ch = wk.tile([128, 1], dt.float32, tag="bnv4", name=f"sch{L}")
                b_ch = wk.tile([128, 1], dt.float32, tag="bnv5", name=f"bch{L}")
                t1 = wk.tile([128, 1], dt.float32, tag="bnv6", name=f"t1{L}")
                nc.vector.tensor_scalar(mean[:fout], stg[:fout, 0:1], 1.0 / N_NODES,
                                        None, OP.mult)
                nc.vector.tensor_scalar(ex2[:fout], stg[:fout, 1:2], 1.0 / N_NODES,
                                        None, OP.mult)
                nc.vector.tensor_tensor(var[:fout], mean[:fout], mean[:fout], OP.mult)
                nc.vector.tensor_tensor(var[:fout], ex2[:fout], var[:fout], OP.subtract)
                nc.vector.tensor_scalar(var[:fout], var[:fout], 1e-5, None, OP.add)
                nc.scalar.activation(sd[:fout], var[:fout], AF.Sqrt)
                nc.vector.reciprocal(s_ch[:fout], sd[:fout])
                nc.vector.tensor_tensor(s_ch[:fout], s_ch[:fout], vg, OP.mult)
                nc.vector.tensor_tensor(t1[:fout], mean[:fout], s_ch[:fout], OP.mult)
                nc.vector.tensor_tensor(b_ch[:fout], vbt, t1[:fout], OP.subtract)
                return s_ch, b_ch

            def apply_bn(L, fout, s_ch, b_ch, to_h3T):
                for t in range(NT):
                    if to_h3T:
                        nc.scalar.activation(
                            h3T[:fout, 128 * t:128 * (t + 1)],
                            z_sb[:fout, 128 * t:128 * (t + 1)],
                            AF.Relu, bias=b_ch[:fout], scale=s_ch[:fout])
                    else:
                        hT = wk.tile([128, 128], dt.float32, tag="hT", name=f"hT{L}_{t}")
                        nc.scalar.activation(
                            hT[:fout, :], z_sb[:fout, 128 * t:128 * (t + 1)],
                            AF.Relu, bias=b_ch[:fout], scale=s_ch[:fout])
                        tp = gp.tile([128, 128], dt.float32, space="PSUM", tag="zp",
                                     name=f"tp{L}_{t}")
                        nc.tensor.transpose(tp[:, :fout], hT[:fout, :],
                                            ident_t[:fout, :fout])
                        nc.vector.tensor_copy(hnat[:, fout * t:fout * (t + 1)],
                                              tp[:, :fout])

            def _emit_regressor(drugT):
                for jc in range(4):
                    f1ps = pp.tile([128, 64], dt.float32, space="PSUM", tag="pp",
                                   name=f"f1ps{jc}")
                    nc.tensor.matmul(f1ps[:], lhsT=f1a_t[:, 128 * jc:128 * (jc + 1)],
                                     rhs=drugT[:], start=True, stop=False)
                    nc.tensor.matmul(f1ps[:], lhsT=f1b_t[:, 128 * jc:128 * (jc + 1)],
                                     rhs=protT[:], start=False, stop=True)
                    nc.scalar.activation(c1T[jc][:, :], f1ps[:], AF.Relu,
                                         bias=fb1_t[:, jc:jc + 1])
                for jc in range(2):
                    f2ps = pp.tile([128, 64], dt.float32, space="PSUM", tag="pp",
                                   name=f"f2ps{jc}")
                    for ic in range(4):
                        nc.tensor.matmul(
                            f2ps[:],
                            lhsT=f2_t[:, 256 * ic + 128 * jc:256 * ic + 128 * jc + 128],
                            rhs=c1T[ic][:, :], start=(ic == 0), stop=(ic == 3))
                    nc.scalar.activation(c2T[jc][:, :], f2ps[:], AF.Relu,
                                         bias=fb2_t[:, jc:jc + 1])
                f3ps = pp.tile([1, 64], dt.float32, space="PSUM", tag="pp", name="f3ps0")
                for ic in range(2):
                    nc.tensor.matmul(f3ps[:], lhsT=f3_t[:, ic:ic + 1],
                                     rhs=c2T[ic][:, :],
                                     start=(ic == 0), stop=(ic == 1))
                outs = wk.tile([1, 64], dt.float32, tag="outs", name="outs0")
                nc.vector.tensor_scalar(outs[:], f3ps[:], fb3_t[:1, 0:1], None, OP.add)
                nc.sync.dma_start(out_d[:], outs[:])

            # ================= emission =================
            if SKIP_GCN:
                for gi in list(pending):
                    protein_group(gi)
                pending.clear()
                drugT0 = wk.tile([128, 64], fr, tag="drugT", name="drugT0")
                nc.vector.memset(drugT0[:].bitcast(dt.float32), 0.0)
                _emit_regressor(drugT0)
            else:
                if pending:
                    protein_group(pending.pop(0))
                s1, bb1 = gcn_layer(0, 64, 5, 64, xpad_d, W1_t)
                apply_bn(0, 64, s1, bb1, False)
                view1 = ag1_in[:, :].rearrange("(t p) j -> p t j", p=128)
                nc.sync.dma_start(view1,
                                  hnat[:, :NT * 64].rearrange("p (t j) -> p t j", j=64))
                nc.gpsimd.collective_compute("AllGather", OP.bypass, replica_groups=rg,
                                             ins=[ag1_in.opt()], outs=[h1_full.opt()])
                filler(1)
                filler(1)

                s2c, bb2 = gcn_layer(1, 64, 64, 128, h1_full, W2_t)
                apply_bn(1, 128, s2c, bb2, False)
                view2 = ag2_in[:, :].rearrange("(t p) j -> p t j", p=128)
                nc.sync.dma_start(view2,
                                  hnat[:, :NT * 128].rearrange("p (t j) -> p t j", j=128))
                nc.gpsimd.collective_compute("AllGather", OP.bypass, replica_groups=rg,
                                             ins=[ag2_in.opt()], outs=[h2_full.opt()])
                filler(1)
                filler(1)

                s3c, bb3 = gcn_layer(2, 128, 128, 128, h2_full, W3_t)
                apply_bn(2, 128, s3c, bb3, True)

                for t in range(NT):
                    tpp = gp.tile([128, 128], dt.float32, space="PSUM", tag="zp",
                                  name=f"tpp_{t}")
                    nc.tensor.transpose(tpp[:], h3T[:, 128 * t:128 * (t + 1)], ident_t[:])
                    nc.vector.tensor_copy(hnat[:, 128 * t:128 * (t + 1)], tpp[:])
                    filler(4)
                for w in range(4):
                    poolw = gp.tile([128, 128], dt.float32, space="PSUM", tag="aggp",
                                    name=f"poolps{w}")
                    for t in range(NT):
                        p1 = wk3.tile([128, 128], fr, tag="p1h",
                                      name=f"p1_{w}_{t}")
                        nc.sync.dma_start(
                            p1[:], P_d[:, (w * NT + t) * 128:(w * NT + t + 1) * 128])
                        nc.tensor.matmul(
                            poolw[:], lhsT=p1[:],
                            rhs=hnat[:, 128 * t:128 * (t + 1)],
                            start=(t == 0), stop=(t == NT - 1))
                    parts = wk.tile([128, 128], dt.float32, tag="parts", name=f"parts{w}")
                    nc.vector.tensor_copy(parts[:], poolw[:])
                    nc.sync.dma_start(arp_in[128 * w:128 * (w + 1), :], parts[:])
                    filler(2)
                nc.gpsimd.collective_compute("ReduceScatter", OP.add, replica_groups=rg,
                                             ins=[arp_in.opt()], outs=[rsp_out.opt()])
                drugsum = wk.tile([64, 128], dt.float32, tag="drugsum", name="drugsum0")
                nc.sync.dma_start(drugsum[:], rsp_out[:])
                drug = wk.tile([64, 128], dt.float32, tag="drug", name="drug0")
                nc.vector.tensor_scalar(drug[:], drugsum[:], cntinv_t[:], None, OP.mult)
                tpd = gp.tile([128, 128], dt.float32, space="PSUM", tag="zp", name="tpd0")
                nc.tensor.transpose(tpd[:, :64], drug[:], ident_t[:64, :64])
                drugT = wk.tile([128, 64], fr, tag="drugT", name="drugT0")
                nc.vector.tensor_copy(drugT[:], tpd[:, :64])

                while pending:
                    protein_group(pending.pop(0))
                _emit_regressor(drugT)

    nc.compile()

    in_maps = []
    for c in range(NCORES):
        in_maps.append({
            "xpad": xpad, "W1": W1, "W2": W2, "W3": W3, "chv": chvec,
            "ident": ident, "pidx": pidx,
            "idxg": idx_w[c], "Sm": S_host[c], "Pm": P_host[c],
            "cntinv": cntinv[64 * c:64 * (c + 1)][:, None],
            "ids4": ids4[c], "embw": emb_w, "sel4": sel4h,
            "ck1T": ck1T, "ck2q": ck2q, "ck3d": ck3d,
            "cb1": cb1[:, None], "cb2": cb2[:, None], "cb3": cb3[:, None],
            "fw1a": fw1a, "fw1b": fw1b, "fb1c": fb1c,
            "fw2p": fw2p, "fb2c": fb2c, "fw3p": fw3p,
            "fb3": np.array([[fb3[0]]], F32),
        })

    res = run_bass_kernel_spmd(nc, in_maps, core_ids=list(range(NCORES)))
    LAST_RES = res
    out = np.concatenate([res.results[c]["out"][0] for c in range(NCORES)])
    return out.astype(F32)



# revision 5
# speedup vs baseline: 1.7062x; 1.7062x over previous
"""DTAModel (drug-target affinity) Trainium2 kernel — 8-core SPMD, single launch.

Sharding: node shards of 4096 (GCN path, dst-sharded edge aggregation with
AllGather of h between layers) + pair shards of 64 (protein CNN + regressor).
All float compute on device; host only builds index/coefficient tables.

v3: bf16 data path end-to-end (1 cyc/col matmuls + fast weight load), self
loops removed from the edge list (device self-term via hT*dis^2), layer-1
gather fully host-prepacked and SBUF-resident, S/P matrices SBUF-resident,
embedding folded into conv1 (rank-26), pool restructured with hnat stationary.
"""
import os
import sys
import types

import numpy as np

N_NODES = 32768
N_EDGES = 131072
BATCH = 512
SEQ_LEN = 1000
VOCAB = 26
EMB = 128
NCORES = 8
NSH = N_NODES // NCORES      # 4096 nodes per core
BSH = BATCH // NCORES        # 64 pairs per core
NT = NSH // 128              # 32 node tiles per core

F32 = np.float32
BF16 = np.dtype("bfloat16") if hasattr(np, "bfloat16") else None
LAST_RES = None


def _bf(x):
    import ml_dtypes
    return np.asarray(x, dtype=ml_dtypes.bfloat16)


def _install_ntff_shim():
    if "antenv.axon_hooks" in sys.modules:
        return
    mod = types.ModuleType("antenv.axon_hooks")
    holder = {"h": None}
    mod.set_axon_ntff_profile_hook = lambda h: holder.__setitem__("h", h)
    mod.get_axon_ntff_profile_hook = lambda: holder["h"]
    sys.modules["antenv.axon_hooks"] = mod
    try:
        from trn_agent_boot.trn_boot import _ntff_profile_via_ctypes
        so = "/opt/axon/libaxon_pjrt.so"
        if os.path.exists(so):
            mod.set_axon_ntff_profile_hook(_ntff_profile_via_ctypes(so))
    except Exception:
        pass


def _prep_edges(edge_index, x):
    """Edge tables without self-loops: per-core dst-sorted 128-edge blocks,
    dense S matrices (bf16), gather idx (int16), and dense layer-1 gathered
    x blocks (pure indexing of the input, done on host)."""
    src = np.asarray(edge_index[0], np.int64)
    dst = np.asarray(edge_index[1], np.int64)
    deg = (1.0 + np.bincount(dst, minlength=N_NODES)).astype(np.float64)
    dis = 1.0 / np.sqrt(deg)
    coef_all = (dis[src] * dis[dst]).astype(F32)
    order = np.argsort(dst, kind="stable")
    s_s, d_s, c_s = src[order], dst[order], coef_all[order]

    tile_of = d_s // 128
    counts = np.bincount(tile_of, minlength=N_NODES // 128)
    bpt = int(np.ceil(counts.max() / 128))
    nblk = NT * bpt

    idx = np.zeros((NCORES, nblk * 128), np.int64)
    dstl = np.zeros((NCORES, nblk * 128), np.int64)
    coef = np.zeros((NCORES, nblk * 128), F32)
    tstart = np.concatenate([[0], np.cumsum(counts)])
    for gt in range(N_NODES // 128):
        c, t = divmod(gt, NT)
        lo, hi = tstart[gt], tstart[gt + 1]
        n = hi - lo
        base = t * bpt * 128
        idx[c, base:base + n] = s_s[lo:hi]
        dstl[c, base:base + n] = d_s[lo:hi] - gt * 128
        coef[c, base:base + n] = c_s[lo:hi]

    # dense S: S_all[c][e, 128*b + j] = coef * (dstl == j)
    S_all = np.zeros((NCORES, nblk, 128, 128), F32)
    bix = np.tile(np.arange(nblk)[:, None], (1, 128)).ravel()
    eix = np.tile(np.arange(128)[None, :], (nblk, 1)).ravel()
    for c in range(NCORES):
        S_all[c][bix, eix, dstl[c]] = coef[c]
    S_all = S_all.transpose(0, 2, 1, 3).reshape(NCORES, 128, nblk * 128)

    nchunk = (nblk * 128 + 2047) // 2048
    idx16 = idx.astype(np.int16)
    wrapped = np.zeros((NCORES, 128, nchunk * 128), np.int16)
    for c in range(NCORES):
        w = idx16[c].reshape(nchunk, 128, 16)
        for ci in range(nchunk):
            blockw = w[ci].reshape(-1, 16).T
            wrapped[c, :, 128 * ci:128 * (ci + 1)] = np.tile(blockw, (8, 1))

    # layer-1 gathered x, dense block layout [128, nblk*8] (pads gather row 0,
    # killed by coef 0 in S)
    xpad8 = np.zeros((N_NODES, 8), F32)
    xpad8[:, :5] = x
    G1 = np.zeros((NCORES, 128, nblk * 8), F32)
    for c in range(NCORES):
        G1[c] = xpad8[idx[c]].reshape(nblk, 128, 8).transpose(1, 0, 2) \
            .reshape(128, nblk * 8)

    dis2 = (dis * dis).astype(F32)
    return bpt, nblk, nchunk, wrapped, S_all, G1, dis2


def kernel(**inputs):
    global LAST_RES
    _install_ntff_shim()
    import concourse.bacc as bacc
    import concourse.tile as tile
    from concourse import hw_specs
    # Tile's static schedule uses this cost model; the default badly
    # underestimates SWDGE gather descriptor generation (~8 ns/desc measured),
    # which starves the PE stream of filler work during gather windows.
    hw_specs.TRN2Spec.SWDGE_NS_PER_DESCRIPTOR = 8.0
    from concourse import mybir
    from concourse.bass_utils import run_bass_kernel_spmd
    from concourse.library_config import mlp as mlp_lib

    g = lambda k: np.ascontiguousarray(np.asarray(inputs[k], F32))
    x = g("x")
    batch = np.asarray(inputs["batch"], np.int64)
    seq = np.asarray(inputs["protein_seq"], np.int64)
    W1 = g("W1")
    W2 = g("W2")
    W3 = g("W3")
    g1, bt1, g2, bt2, g3, bt3 = g("g1"), g("bt1"), g("g2"), g("bt2"), g("g3"), g("bt3")
    emb_w = g("emb")
    ck1, cb1, ck2, cb2, ck3, cb3 = g("ck1"), g("cb1"), g("ck2"), g("cb2"), g("ck3"), g("cb3")
    fw1, fb1, fw2, fb2, fw3, fb3 = g("fw1"), g("fb1"), g("fw2"), g("fb2"), g("fw3"), g("fb3")

    # ---------------- host tables ----------------
    bpt, nblk, nchunk, idx_w, S_host, G1_host, dis2 = _prep_edges(
        np.asarray(inputs["edge_index"]), x)

    cnt = np.bincount(batch, minlength=BATCH).astype(F32)
    cntinv = (1.0 / np.maximum(cnt, 1.0)).astype(F32)
    # pooling one-hots: P_all[c][n, (w*NT + t)*128 + j] = (batch[node] == 128w + j)
    P_host = np.zeros((NCORES, 128, 4 * NT * 128), F32)
    for c in range(NCORES):
        bl = batch[NSH * c:NSH * (c + 1)].reshape(NT, 128)
        for t in range(NT):
            w = bl[t] // 128
            j = bl[t] % 128
            P_host[c, np.arange(128), (w * NT + t) * 128 + j] = 1.0

    chvec = np.zeros((128, 6), F32)
    chvec[:64, 0], chvec[:64, 1] = g1, bt1
    chvec[:, 2], chvec[:, 3] = g2, bt2
    chvec[:, 4], chvec[:, 5] = g3, bt3

    ident = np.eye(128, dtype=F32)
    # per-core self-term inputs: x^T (padded to 8 rows) and dis^2 broadcast
    xT8 = np.zeros((NCORES, 8, NSH), F32)
    d2b = np.zeros((NCORES, 128, NSH), F32)
    for c in range(NCORES):
        xT8[c, :5, :] = x[NSH * c:NSH * (c + 1)].T
        d2b[c, :, :] = dis2[NSH * c:NSH * (c + 1)][None, :]

    ids4 = seq.astype(F32).reshape(NCORES, BSH // 4, 4, SEQ_LEN).transpose(0, 2, 1, 3) \
        .reshape(NCORES, 4, (BSH // 4) * SEQ_LEN).copy()
    sel4h = np.zeros((4, 4 * VOCAB), F32)
    for s in range(4):
        sel4h[s, VOCAB * s:VOCAB * (s + 1)] = 1.0
    # pidx bands: partition 32*s + v compares against v (pad rows never read)
    pidx128 = np.zeros((128, 1), F32)
    for s in range(4):
        pidx128[32 * s:32 * s + 32, 0] = np.arange(32)
    # conv1 with embedding folded in (rank 26): W1f[32*s+v, 32*t+o] =
    # (emb @ ck1[:, :, t].T)[v, o], replicated across the 4 sequence bands
    W1f = np.zeros((128, 128), F32)
    for t in range(4):
        fold = emb_w @ ck1[:, :, t].T          # [26, 32]
        for s in range(4):
            W1f[32 * s:32 * s + 26, 32 * t:32 * t + 32] = fold
    cb1rep = np.tile(cb1, 4)[:, None]
    ck2q = np.concatenate([np.tile(ck2[:, :, t].T, (4, 1)) for t in range(6)], axis=1)
    ck3d = np.concatenate([np.tile(ck3[:, :, t].T, (2, 1)) for t in range(8)], axis=1)
    fw1a = np.ascontiguousarray(fw1[:128])
    fw1b = np.ascontiguousarray(fw1[128:224])
    fb1c = np.ascontiguousarray(fb1.reshape(4, 128).T)
    fw2p = np.ascontiguousarray(fw2.reshape(4, 128, 256).transpose(1, 0, 2).reshape(128, 1024))
    fb2c = np.ascontiguousarray(fb2.reshape(2, 128).T)
    fw3p = np.ascontiguousarray(fw3.reshape(2, 128).T)

    # ---------------- device program ----------------
    dt = mybir.dt
    AF = mybir.ActivationFunctionType
    OP = mybir.AluOpType
    rg = [list(range(NCORES))]
    bf = dt.bfloat16

    nc = bacc.Bacc("TRN2", target_bir_lowering=False, debug=False, num_devices=NCORES)

    def din(name, shape, dty=dt.float32):
        return nc.dram_tensor(name, list(shape), dty, kind="ExternalInput")

    W1_d, W2_d, W3_d = din("W1", [5, 64], bf), din("W2", [64, 128], bf), din("W3", [128, 128], bf)
    chv_d = din("chv", [128, 6])
    ident_d = din("ident", [128, 128], bf)
    identf_d = din("identf", [64, 64])
    pidx_d = din("pidx", [128, 1])
    idx_d = din("idxg", [128, nchunk * 128], dt.int16)
    S_d = din("Sm", [128, nblk * 128], bf)
    G1_d = din("G1m", [128, nblk * 8], bf)
    P_d = din("Pm", [128, 4 * NT * 128], bf)
    xT8_d = din("xT8", [8, NSH], bf)
    d2b_d = din("d2b", [128, NSH], bf)
    cntinv_d = din("cntinv", [64, 1])
    ids_d = din("ids4", [4, (BSH // 4) * SEQ_LEN], bf)
    sel_d = din("sel4", [4, 4 * VOCAB], bf)
    W1f_d = din("W1f", [128, 128], bf)
    ck2_d, ck3_d = din("ck2q", [128, 384], bf), din("ck3d", [128, 768], bf)
    cb1_d, cb2_d, cb3_d = din("cb1r", [128, 1]), din("cb2", [64, 1]), din("cb3", [96, 1])
    f1a_d, f1b_d = din("fw1a", [128, 512], bf), din("fw1b", [96, 512], bf)
    fb1_d = din("fb1c", [128, 4])
    f2_d, fb2_d = din("fw2p", [128, 1024], bf), din("fb2c", [128, 2])
    f3_d, fb3_d = din("fw3p", [128, 2], bf), din("fb3", [1, 1])
    out_d = nc.dram_tensor("out", [1, BSH], dt.float32, kind="ExternalOutput")

    with tile.TileContext(nc) as tc:
        with (
            tc.tile_pool(name="dram", bufs=1, space="DRAM") as dram,
            tc.tile_pool(name="cst", bufs=1) as cst,
            tc.tile_pool(name="big", bufs=1) as big,
            tc.tile_pool(name="wk", bufs=2) as wk,
            tc.tile_pool(name="wk1", bufs=1) as wk1,
            tc.tile_pool(name="gp", bufs=2, space="PSUM") as gp,
            tc.tile_pool(name="pp", bufs=4, space="PSUM") as pp,
        ):
            nc.gpsimd.load_library(mlp_lib)

            ag1_in = dram.tile([NSH, 128], bf)
            h1_full = dram.tile([N_NODES, 128], bf, addr_space="Shared")
            ag2_in = dram.tile([NSH, 128], bf)
            h2_full = dram.tile([N_NODES, 128], bf, addr_space="Shared")
            ar_ins = [dram.tile([128, 2], dt.float32, name=f"arin{i}") for i in range(3)]
            ar_outs = [dram.tile([128, 2], dt.float32, addr_space="Shared",
                                 name=f"arout{i}") for i in range(3)]
            arp_in = dram.tile([BATCH, 128], dt.float32)
            rsp_out = dram.tile([BSH, 128], dt.float32)

            def load(name, d, shape, dty=dt.float32):
                t = cst.tile(shape, dty, name=name)
                nc.sync.dma_start(t[:], d[:])
                return t

            ident_t = load("ident_t", ident_d, [128, 128], bf)
            identf_t = load("identf_t", identf_d, [64, 64])
            pidx_t = load("pidx_t", pidx_d, [128, 1])
            chv_t = load("chv_t", chv_d, [128, 6])
            W1_t = load("W1_t", W1_d, [5, 64], bf)
            W2_t = load("W2_t", W2_d, [64, 128], bf)
            W3_t = load("W3_t", W3_d, [128, 128], bf)
            idx_t = load("idx_t", idx_d, [128, nchunk * 128], dt.int16)
            cntinv_t = load("cntinv_t", cntinv_d, [64, 1])
            ids_t = load("ids_t", ids_d, [4, (BSH // 4) * SEQ_LEN], bf)
            sel_t = load("sel_t", sel_d, [4, 4 * VOCAB], bf)
            W1f_t = load("W1f_t", W1f_d, [128, 128], bf)
            ck2_t = load("ck2_t", ck2_d, [128, 384], bf)
            ck3_t = load("ck3_t", ck3_d, [128, 768], bf)
            cb1_t = load("cb1_t", cb1_d, [128, 1])
            cb2_t = load("cb2_t", cb2_d, [64, 1])
            cb3_t = load("cb3_t", cb3_d, [96, 1])
            f1a_t = load("f1a_t", f1a_d, [128, 512], bf)
            f1b_t = load("f1b_t", f1b_d, [96, 512], bf)
            fb1_t = load("fb1_t", fb1_d, [128, 4])
            f2_t = load("f2_t", f2_d, [128, 1024], bf)
            fb2_t = load("fb2_t", fb2_d, [128, 2])
            f3_t = load("f3_t", f3_d, [128, 2], bf)
            fb3_t = load("fb3_t", fb3_d, [1, 1])
            xT8_t = load("xT8_t", xT8_d, [8, NSH], bf)
            d2b_t = load("d2b_t", d2b_d, [128, NSH], bf)
            S_sb = load("S_sb", S_d, [128, nblk * 128], bf)
            G1_sb = load("G1_sb", G1_d, [128, nblk * 8], bf)
            P_sb = load("P_sb", P_d, [128, 4 * NT * 128], bf)

            z_sb = big.tile([128, NSH], bf)
            hT_sb = big.tile([128, NSH], bf)
            selfd = big.tile([128, NSH], bf)
            hnat = big.tile([128, NSH], bf)
            protT = big.tile([96, BSH], bf)
            c1T = [big.tile([128, BSH], bf, name=f"c1T_{j}") for j in range(4)]
            c2T = [big.tile([128, BSH], bf, name=f"c2T_{j}") for j in range(2)]

            nc.vector.memset(hnat[:], 0.0)
            # layer-1 self term: x^T * dis^2
            nc.vector.tensor_tensor(selfd[:8, :], xT8_t[:], d2b_t[:8, :], OP.mult)

            # ---------------- protein group ----------------
            def protein_group(gi):
                # one-hot of the 4 sequences, banded [32s+v], cols 1..1000
                oh = wk.tile([128, 1002], bf, tag="oh", name=f"oh{gi}")
                nc.vector.memset(oh[:, 0:1], 0.0)
                nc.vector.memset(oh[:, 1001:1002], 0.0)
                for (l0, Lc) in ((0, 512), (512, 488)):
                    pb = pp.tile([128, 512], dt.float32, space="PSUM", tag="pp",
                                 name=f"pb{gi}_{l0}")
                    for si in range(4):
                        nc.tensor.matmul(
                            pb[32 * si:32 * si + 26, :Lc],
                            lhsT=sel_t[:, VOCAB * si:VOCAB * (si + 1)],
                            rhs=ids_t[:, SEQ_LEN * gi + l0:SEQ_LEN * gi + l0 + Lc],
                            start=True, stop=True, tile_position=(0, 32 * si))
                    nc.vector.tensor_scalar(oh[:, 1 + l0:1 + l0 + Lc], pb[:, :Lc],
                                            pidx_t[:], None, OP.is_equal)
                h1Q = wk.tile([128, 1003], bf, tag="h1Q", name=f"h1Q{gi}")
                nc.vector.memset(h1Q[:, 0:2], 0.0)
                nc.vector.memset(h1Q[:, 1001:1003], 0.0)
                for (l0, Lc) in ((0, 512), (512, 487)):
                    c1p = pp.tile([128, 512], dt.float32, space="PSUM", tag="pp",
                                  name=f"c1p{gi}_{l0}")
                    for tap in range(4):
                        for si in range(4):
                            nc.tensor.matmul(
                                c1p[32 * si:32 * (si + 1), :Lc],
                                lhsT=W1f_t[32 * si:32 * si + 26,
                                           32 * tap:32 * (tap + 1)],
                                rhs=oh[32 * si:32 * si + 26, l0 + tap:l0 + tap + Lc],
                                start=(tap == 0), stop=(tap == 3),
                                tile_position=(32 * si, 32 * si))
                    nc.scalar.activation(h1Q[:, 2 + l0:2 + l0 + Lc], c1p[:, :Lc],
                                         AF.Relu, bias=cb1_t[:])
                h2D = []
                for p in range(2):
                    h2p = wk.tile([128, 1005], bf, tag=f"h2D{p}",
                                  name=f"h2D{gi}_{p}")
                    nc.vector.memset(h2p[:, 0:3], 0.0)
                    nc.vector.memset(h2p[:, 1001:1005], 0.0)
                    h2D.append(h2p)
                for (l0, Lc) in ((0, 512), (512, 486)):
                    c2ps = []
                    for si in range(4):
                        c2p = pp.tile([64, 512], dt.float32, space="PSUM", tag="pp",
                                      name=f"c2p{gi}_{l0}_{si}")
                        c2ps.append(c2p)
                    for tap in range(6):
                        for si in range(4):
                            nc.tensor.matmul(
                                c2ps[si][:, :Lc],
                                lhsT=ck2_t[32 * si:32 * (si + 1),
                                           64 * tap:64 * (tap + 1)],
                                rhs=h1Q[32 * si:32 * (si + 1), l0 + tap:l0 + tap + Lc],
                                start=(tap == 0), stop=(tap == 5),
                                tile_position=(32 * si, 0))
                    for si in range(4):
                        nc.scalar.activation(
                            h2D[si // 2][64 * (si % 2):64 * (si % 2 + 1),
                                         3 + l0:3 + l0 + Lc],
                            c2ps[si][:, :Lc], AF.Relu, bias=cb2_t[:])
                for p in range(2):
                    mx = [wk.tile([96, 1], dt.float32, tag=f"mx{j}",
                                  name=f"mx{gi}_{p}_{j}") for j in range(2)]
                    tmp = [wk.tile([96, 1], dt.float32, tag=f"tm{j}",
                                   name=f"tm{gi}_{p}_{j}") for j in range(2)]
                    for (l0, Lc) in ((0, 512), (512, 486)):
                        Lreal = 512 if l0 == 0 else 485
                        c3ps = []
                        for j in range(2):
                            c3p = pp.tile([96, 512], dt.float32, space="PSUM", tag="pp",
                                          name=f"c3p{gi}_{p}_{l0}_{j}")
                            c3ps.append(c3p)
                        for tap in range(8):
                            for j in range(2):
                                nc.tensor.matmul(
                                    c3ps[j][:, :Lc],
                                    lhsT=ck3_t[64 * j:64 * (j + 1),
                                               96 * tap:96 * (tap + 1)],
                                    rhs=h2D[p][64 * j:64 * (j + 1),
                                               l0 + tap:l0 + tap + Lc],
                                    start=(tap == 0), stop=(tap == 7),
                                    tile_position=(64 * j, 0))
                        for j in range(2):
                            dst = mx[j] if l0 == 0 else tmp[j]
                            nc.vector.tensor_reduce(dst[:], c3ps[j][:, :Lreal],
                                                    axis=mybir.AxisListType.X, op=OP.max)
                            if l0 != 0:
                                nc.vector.tensor_tensor(mx[j][:], mx[j][:], tmp[j][:],
                                                        OP.max)
                    for j in range(2):
                        s_idx = 4 * gi + 2 * p + j
                        nc.scalar.activation(protT[:, s_idx:s_idx + 1], mx[j][:],
                                             AF.Relu, bias=cb3_t[:])

            pending = list(range(16))
            slot = [0]

            def filler(period=8):
                slot[0] += 1
                if pending and slot[0] % period == 0:
                    protein_group(pending.pop(0))

            # ---------------- GCN layer ----------------
            def gcn_layer(L, fin, fout, src_dram, Wt, period):
                zs = cst.tile([128, NT], dt.float32, name=f"zs{L}")
                zq = cst.tile([128, NT], dt.float32, name=f"zq{L}")
                sq_scr = wk.tile([128, 128], dt.float32, tag="sqs", name=f"sqs{L}")
                Gt = None
                for t in range(NT):
                    aggT = gp.tile([128, 128], dt.float32, space="PSUM", tag="aggp",
                                   name=f"agg{L}_{t}")
                    for k in range(bpt):
                        b = t * bpt + k
                        if L == 0:
                            lhsT = G1_sb[:, 8 * b:8 * b + 5]
                        else:
                            ci, bb = divmod(b, 16)
                            if bb == 0:
                                Gt = wk.tile([128, 16, 128], bf, tag="gch",
                                             name=f"g{L}_{ci}")
                                nc.gpsimd.dma_gather(
                                    Gt[:], src_dram[:],
                                    idx_t[:, 128 * ci:128 * (ci + 1)], 2048, 2048,
                                    128, single_packet=False)
                            lhsT = Gt[:, bb, :fin]
                        nc.tensor.matmul(aggT[:fin, :], lhsT=lhsT,
                                         rhs=S_sb[:, 128 * b:128 * (b + 1)],
                                         start=(k == 0), stop=(k == bpt - 1))
                    aggS = wk.tile([fin, 128], bf, tag="aggS",
                                   name=f"aggS{L}_{t}")
                    nc.vector.tensor_tensor(aggS[:], aggT[:fin, :],
                                            selfd[:fin, 128 * t:128 * (t + 1)],
                                            OP.add)
                    zT = gp.tile([128, 128], dt.float32, space="PSUM", tag="zp",
                                 name=f"z{L}_{t}")
                    nc.tensor.matmul(zT[:fout, :], lhsT=Wt[:fin, :fout], rhs=aggS[:],
                                     start=True, stop=True)
                    nc.scalar.activation(z_sb[:fout, 128 * t:128 * (t + 1)], zT[:fout, :],
                                         AF.Copy, accum_out=zs[:fout, t:t + 1])
                    nc.scalar.activation(sq_scr[:fout, :], zT[:fout, :], AF.Square,
                                         accum_out=zq[:fout, t:t + 1])
                    filler(period)
                ssum = wk.tile([128, 2], dt.float32, tag="ssum", name=f"ssum{L}")
                nc.vector.memset(ssum[:], 0.0)
                nc.vector.tensor_reduce(ssum[:fout, 0:1], zs[:fout, :],
                                        axis=mybir.AxisListType.X, op=OP.add)
                nc.vector.tensor_reduce(ssum[:fout, 1:2], zq[:fout, :],
                                        axis=mybir.AxisListType.X, op=OP.add)
                nc.sync.dma_start(ar_ins[L][:], ssum[:])
                nc.gpsimd.collective_compute(
                    "AllReduce", OP.add, replica_groups=rg,
                    ins=[ar_ins[L].opt()], outs=[ar_outs[L].opt()])
                stg = wk.tile([128, 2], dt.float32, tag="stg", name=f"stg{L}")
                nc.sync.dma_start(stg[:], ar_outs[L][:])
                vg = chv_t[:fout, 2 * L:2 * L + 1]
                vbt = chv_t[:fout, 2 * L + 1:2 * L + 2]
                mean = wk.tile([128, 1], dt.float32, tag="bnv0", name=f"mean{L}")
                ex2 = wk.tile([128, 1], dt.float32, tag="bnv1", name=f"ex2{L}")
                var = wk.tile([128, 1], dt.float32, tag="bnv2", name=f"var{L}")
                sd = wk.tile([128, 1], dt.float32, tag="bnv3", name=f"sd{L}")
                s_ch = wk.tile([128, 1], dt.float32, tag="bnv4", name=f"sch{L}")
                b_ch = wk.tile([128, 1], dt.float32, tag="bnv5", name=f"bch{L}")
                t1 = wk.tile([128, 1], dt.float32, tag="bnv6", name=f"t1{L}")
                nc.vector.tensor_scalar(mean[:fout], stg[:fout, 0:1], 1.0 / N_NODES,
                                        None, OP.mult)
                nc.vector.tensor_scalar(ex2[:fout], stg[:fout, 1:2], 1.0 / N_NODES,
                                        None, OP.mult)
                nc.vector.tensor_tensor(var[:fout], mean[:fout], mean[:fout], OP.mult)
                nc.vector.tensor_tensor(var[:fout], ex2[:fout], var[:fout], OP.subtract)
                nc.vector.tensor_scalar(var[:fout], var[:fout], 1e-5, None, OP.add)
                nc.scalar.activation(sd[:fout], var[:fout], AF.Sqrt)
                nc.vector.reciprocal(s_ch[:fout], sd[:fout])
                nc.vector.tensor_tensor(s_ch[:fout], s_ch[:fout], vg, OP.mult)
                nc.vector.tensor_tensor(t1[:fout], mean[:fout], s_ch[:fout], OP.mult)
                nc.vector.tensor_tensor(b_ch[:fout], vbt, t1[:fout], OP.subtract)
                return s_ch, b_ch

            def apply_bn(L, fout, s_ch, b_ch):
                for t in range(NT):
                    nc.scalar.activation(
                        hT_sb[:fout, 128 * t:128 * (t + 1)],
                        z_sb[:fout, 128 * t:128 * (t + 1)],
                        AF.Relu, bias=b_ch[:fout], scale=s_ch[:fout])
                    tp = gp.tile([128, 128], bf, space="PSUM", tag="zp",
                                 name=f"tp{L}_{t}")
                    nc.tensor.transpose(tp[:, :fout], hT_sb[:fout, 128 * t:128 * (t + 1)],
                                        ident_t[:fout, :fout])
                    nc.vector.tensor_copy(hnat[:, 128 * t:128 * t + fout],
                                          tp[:, :fout])
                if L < 2:
                    nc.vector.tensor_tensor(selfd[:fout, :], hT_sb[:fout, :],
                                            d2b_t[:fout, :], OP.mult)

            def _emit_regressor(drugT):
                for jc in range(4):
                    f1ps = pp.tile([128, 64], dt.float32, space="PSUM", tag="pp",
                                   name=f"f1ps{jc}")
                    nc.tensor.matmul(f1ps[:], lhsT=f1a_t[:, 128 * jc:128 * (jc + 1)],
                                     rhs=drugT[:], start=True, stop=False)
                    nc.tensor.matmul(f1ps[:], lhsT=f1b_t[:, 128 * jc:128 * (jc + 1)],
                                     rhs=protT[:], start=False, stop=True)
                    nc.scalar.activation(c1T[jc][:, :], f1ps[:], AF.Relu,
                                         bias=fb1_t[:, jc:jc + 1])
                for jc in range(2):
                    f2ps = pp.tile([128, 64], dt.float32, space="PSUM", tag="pp",
                                   name=f"f2ps{jc}")
                    for ic in range(4):
                        nc.tensor.matmul(
                            f2ps[:],
                            lhsT=f2_t[:, 256 * ic + 128 * jc:256 * ic + 128 * jc + 128],
                            rhs=c1T[ic][:, :], start=(ic == 0), stop=(ic == 3))
                    nc.scalar.activation(c2T[jc][:, :], f2ps[:], AF.Relu,
                                         bias=fb2_t[:, jc:jc + 1])
                f3ps = pp.tile([1, 64], dt.float32, space="PSUM", tag="pp", name="f3ps0")
                for ic in range(2):
                    nc.tensor.matmul(f3ps[:], lhsT=f3_t[:, ic:ic + 1],
                                     rhs=c2T[ic][:, :],
                                     start=(ic == 0), stop=(ic == 1))
                outs = wk.tile([1, 64], dt.float32, tag="outs", name="outs0")
                nc.vector.tensor_scalar(outs[:], f3ps[:], fb3_t[:1, 0:1], None, OP.add)
                nc.sync.dma_start(out_d[:], outs[:])

            # ================= emission =================
            s1, bb1 = gcn_layer(0, 5, 64, None, W1_t, 99)
            filler(1)
            apply_bn(0, 64, s1, bb1)
            view1 = ag1_in[:, :].rearrange("(t p) j -> p t j", p=128)
            nc.sync.dma_start(view1,
                              hnat[:, :].rearrange("p (t j) -> p t j", j=128))
            nc.gpsimd.collective_compute("AllGather", OP.bypass, replica_groups=rg,
                                         ins=[ag1_in.opt()], outs=[h1_full.opt()])
            filler(1)
            filler(1)

            s2c, bb2 = gcn_layer(1, 64, 128, h1_full, W2_t, 6)
            filler(1)
            apply_bn(1, 128, s2c, bb2)
            view2 = ag2_in[:, :].rearrange("(t p) j -> p t j", p=128)
            nc.sync.dma_start(view2,
                              hnat[:, :].rearrange("p (t j) -> p t j", j=128))
            nc.gpsimd.collective_compute("AllGather", OP.bypass, replica_groups=rg,
                                         ins=[ag2_in.opt()], outs=[h2_full.opt()])
            filler(1)
            filler(1)

            s3c, bb3 = gcn_layer(2, 128, 128, h2_full, W3_t, 6)
            filler(1)
            apply_bn(2, 128, s3c, bb3)
            while pending:
                protein_group(pending.pop(0))

            # ---------------- pooling ----------------
            poolps = [pp.tile([128, 128], dt.float32, space="PSUM", tag="pp",
                              name=f"poolps{w}") for w in range(4)]
            for t in range(NT):
                for w in range(4):
                    nc.tensor.matmul(
                        poolps[w][:], lhsT=hnat[:, 128 * t:128 * (t + 1)],
                        rhs=P_sb[:, (w * NT + t) * 128:(w * NT + t + 1) * 128],
                        start=(t == 0), stop=(t == NT - 1))
            for w in range(4):
                poolS = wk.tile([128, 128], bf, tag="poolS", name=f"poolS{w}")
                nc.vector.tensor_copy(poolS[:], poolps[w][:])
                tpw = gp.tile([128, 128], bf, space="PSUM", tag="zp", name=f"tpw{w}")
                nc.tensor.transpose(tpw[:], poolS[:], ident_t[:])
                parts = wk.tile([128, 128], dt.float32, tag="parts", name=f"parts{w}")
                nc.vector.tensor_copy(parts[:], tpw[:])
                nc.sync.dma_start(arp_in[128 * w:128 * (w + 1), :], parts[:])
            nc.gpsimd.collective_compute("ReduceScatter", OP.add, replica_groups=rg,
                                         ins=[arp_in.opt()], outs=[rsp_out.opt()])
            drugsum = wk.tile([64, 128], dt.float32, tag="drugsum", name="drugsum0")
            nc.sync.dma_start(drugsum[:], rsp_out[:])
            drug = wk.tile([64, 128], dt.float32, tag="drug", name="drug0")
            nc.vector.tensor_scalar(drug[:], drugsum[:], cntinv_t[:], None, OP.mult)
            tpd = gp.tile([128, 128], dt.float32, space="PSUM", tag="zp", name="tpd0")
            nc.tensor.transpose(tpd[:, :64], drug[:], identf_t[:])
            drugT = wk.tile([128, 64], bf, tag="drugT", name="drugT0")
            nc.vector.tensor_copy(drugT[:], tpd[:, :64])

            _emit_regressor(drugT)

    nc.compile()

    in_maps = []
    for c in range(NCORES):
        in_maps.append({
            "W1": _bf(W1), "W2": _bf(W2), "W3": _bf(W3), "chv": chvec,
            "ident": _bf(ident), "identf": ident[:64, :64].copy(),
            "pidx": pidx128,
            "idxg": idx_w[c], "Sm": _bf(S_host[c]), "G1m": _bf(G1_host[c]),
            "Pm": _bf(P_host[c]),
            "xT8": _bf(xT8[c]), "d2b": _bf(d2b[c]),
            "cntinv": cntinv[64 * c:64 * (c + 1)][:, None],
            "ids4": _bf(ids4[c]), "sel4": _bf(sel4h),
            "W1f": _bf(W1f), "ck2q": _bf(ck2q), "ck3d": _bf(ck3d),
            "cb1r": cb1rep, "cb2": cb2[:, None], "cb3": cb3[:, None],
            "fw1a": _bf(fw1a), "fw1b": _bf(fw1b), "fb1c": fb1c,
            "fw2p": _bf(fw2p), "fb2c": fb2c, "fw3p": _bf(fw3p),
            "fb3": np.array([[fb3[0]]], F32),
        })

    res = run_bass_kernel_spmd(nc, in_maps, core_ids=list(range(NCORES)))
    LAST_RES = res
    out = np.concatenate([res.results[c]["out"][0] for c in range(NCORES)])
    return out.astype(F32)


# revision 18
# speedup vs baseline: 1.8299x; 1.0725x over previous
"""DTAModel (drug-target affinity) Trainium2 kernel — 8-core SPMD, single launch.

Sharding: node shards of 4096 (GCN path, dst-sharded edge aggregation with
AllGather of h between layers) + pair shards of 64 (protein CNN + regressor).
All float compute on device; host only builds index/coefficient tables.

v3: bf16 data path end-to-end (1 cyc/col matmuls + fast weight load), self
loops removed from the edge list (device self-term via hT*dis^2), layer-1
gather fully host-prepacked and SBUF-resident, S/P matrices SBUF-resident,
embedding folded into conv1 (rank-26), pool restructured with hnat stationary.
"""
import os
import sys
import types

import numpy as np

N_NODES = 32768
N_EDGES = 131072
BATCH = 512
SEQ_LEN = 1000
VOCAB = 26
EMB = 128
NCORES = 8
NSH = N_NODES // NCORES      # 4096 nodes per core
BSH = BATCH // NCORES        # 64 pairs per core
NT = NSH // 128              # 32 node tiles per core

F32 = np.float32
BF16 = np.dtype("bfloat16") if hasattr(np, "bfloat16") else None
LAST_RES = None


def _bf(x):
    import ml_dtypes
    return np.asarray(x, dtype=ml_dtypes.bfloat16)


def _install_ntff_shim():
    if "antenv.axon_hooks" in sys.modules:
        return
    mod = types.ModuleType("antenv.axon_hooks")
    holder = {"h": None}
    mod.set_axon_ntff_profile_hook = lambda h: holder.__setitem__("h", h)
    mod.get_axon_ntff_profile_hook = lambda: holder["h"]
    sys.modules["antenv.axon_hooks"] = mod
    try:
        from trn_agent_boot.trn_boot import _ntff_profile_via_ctypes
        so = "/opt/axon/libaxon_pjrt.so"
        if os.path.exists(so):
            mod.set_axon_ntff_profile_hook(_ntff_profile_via_ctypes(so))
    except Exception:
        pass


def _prep_edges(edge_index, x):
    """Edge tables without self-loops: per-core dst-sorted 128-edge blocks,
    dense S matrices (bf16), gather idx (int16), and dense layer-1 gathered
    x blocks (pure indexing of the input, done on host)."""
    src = np.asarray(edge_index[0], np.int64)
    dst = np.asarray(edge_index[1], np.int64)
    deg = (1.0 + np.bincount(dst, minlength=N_NODES)).astype(np.float64)
    dis = 1.0 / np.sqrt(deg)
    coef_all = (dis[src] * dis[dst]).astype(F32)
    order = np.argsort(dst, kind="stable")
    s_s, d_s, c_s = src[order], dst[order], coef_all[order]

    tile_of = d_s // 128
    counts = np.bincount(tile_of, minlength=N_NODES // 128)
    bpt = int(np.ceil(counts.max() / 128))
    nblk = NT * bpt

    idx = np.zeros((NCORES, nblk * 128), np.int64)
    dstl = np.zeros((NCORES, nblk * 128), np.int64)
    coef = np.zeros((NCORES, nblk * 128), F32)
    tstart = np.concatenate([[0], np.cumsum(counts)])
    for gt in range(N_NODES // 128):
        c, t = divmod(gt, NT)
        lo, hi = tstart[gt], tstart[gt + 1]
        n = hi - lo
        base = t * bpt * 128
        idx[c, base:base + n] = s_s[lo:hi]
        dstl[c, base:base + n] = d_s[lo:hi] - gt * 128
        coef[c, base:base + n] = c_s[lo:hi]

    # dense S: S_all[c][e, 128*b + j] = coef * (dstl == j)
    S_all = np.zeros((NCORES, nblk, 128, 128), F32)
    bix = np.tile(np.arange(nblk)[:, None], (1, 128)).ravel()
    eix = np.tile(np.arange(128)[None, :], (nblk, 1)).ravel()
    for c in range(NCORES):
        S_all[c][bix, eix, dstl[c]] = coef[c]
    S_all = S_all.transpose(0, 2, 1, 3).reshape(NCORES, 128, nblk * 128)

    nchunk = (nblk * 128 + 2047) // 2048
    idx16 = idx.astype(np.int16)
    wrapped = np.zeros((NCORES, 128, nchunk * 128), np.int16)
    for c in range(NCORES):
        w = idx16[c].reshape(nchunk, 128, 16)
        for ci in range(nchunk):
            blockw = w[ci].reshape(-1, 16).T
            wrapped[c, :, 128 * ci:128 * (ci + 1)] = np.tile(blockw, (8, 1))

    # layer-1 gathered x, dense block layout [128, nblk*8] (pads gather row 0,
    # killed by coef 0 in S)
    xpad8 = np.zeros((N_NODES, 8), F32)
    xpad8[:, :5] = x
    G1 = np.zeros((NCORES, 128, nblk * 8), F32)
    for c in range(NCORES):
        G1[c] = xpad8[idx[c]].reshape(nblk, 128, 8).transpose(1, 0, 2) \
            .reshape(128, nblk * 8)

    dis2 = (dis * dis).astype(F32)
    return bpt, nblk, nchunk, wrapped, S_all, G1, dis2


def kernel(**inputs):
    global LAST_RES
    _install_ntff_shim()
    import concourse.bacc as bacc
    import concourse.tile as tile
    from concourse import hw_specs
    # Tile's static schedule uses this cost model; the default badly
    # underestimates SWDGE gather descriptor generation (~8 ns/desc measured),
    # which starves the PE stream of filler work during gather windows.
    hw_specs.TRN2Spec.SWDGE_NS_PER_DESCRIPTOR = 8.0
    from concourse import mybir
    from concourse.bass_utils import run_bass_kernel_spmd
    from concourse.library_config import mlp as mlp_lib

    g = lambda k: np.ascontiguousarray(np.asarray(inputs[k], F32))
    x = g("x")
    batch = np.asarray(inputs["batch"], np.int64)
    seq = np.asarray(inputs["protein_seq"], np.int64)
    W1 = g("W1")
    W2 = g("W2")
    W3 = g("W3")
    g1, bt1, g2, bt2, g3, bt3 = g("g1"), g("bt1"), g("g2"), g("bt2"), g("g3"), g("bt3")
    emb_w = g("emb")
    ck1, cb1, ck2, cb2, ck3, cb3 = g("ck1"), g("cb1"), g("ck2"), g("cb2"), g("ck3"), g("cb3")
    fw1, fb1, fw2, fb2, fw3, fb3 = g("fw1"), g("fb1"), g("fw2"), g("fb2"), g("fw3"), g("fb3")

    # ---------------- host tables ----------------
    bpt, nblk, nchunk, idx_w, S_host, G1_host, dis2 = _prep_edges(
        np.asarray(inputs["edge_index"]), x)

    cnt = np.bincount(batch, minlength=BATCH).astype(F32)
    cntinv = (1.0 / np.maximum(cnt, 1.0)).astype(F32)
    # pooling one-hots, t-major: P_all[c][n, t*512 + batch[node]] (batch < 512)
    P_host = np.zeros((NCORES, 128, NT * 512), F32)
    for c in range(NCORES):
        bl = batch[NSH * c:NSH * (c + 1)].reshape(NT, 128)
        for t in range(NT):
            P_host[c, np.arange(128), t * 512 + bl[t]] = 1.0

    chvec = np.zeros((128, 6), F32)
    chvec[:64, 0], chvec[:64, 1] = g1, bt1
    chvec[:, 2], chvec[:, 3] = g2, bt2
    chvec[:, 4], chvec[:, 5] = g3, bt3

    ident = np.eye(128, dtype=F32)
    # per-core self-term inputs: x^T (padded to 8 rows) and dis^2 broadcast
    xT8 = np.zeros((NCORES, 8, NSH), F32)
    d2b = np.zeros((NCORES, 128, NSH), F32)
    for c in range(NCORES):
        xT8[c, :5, :] = x[NSH * c:NSH * (c + 1)].T
        d2b[c, :, :] = dis2[NSH * c:NSH * (c + 1)][None, :]

    ids4 = seq.astype(F32).reshape(NCORES, BSH // 4, 4, SEQ_LEN).transpose(0, 2, 1, 3) \
        .reshape(NCORES, 4, (BSH // 4) * SEQ_LEN).copy()
    sel4h = np.zeros((4, 4 * VOCAB), F32)
    for s in range(4):
        sel4h[s, VOCAB * s:VOCAB * (s + 1)] = 1.0
    # pidx bands: partition 32*s + v compares against v (pad rows never read)
    pidx128 = np.zeros((128, 1), F32)
    for s in range(4):
        pidx128[32 * s:32 * s + 32, 0] = np.arange(32)
    # conv1 with embedding folded in (rank 26): W1f[32*s+v, 32*t+o] =
    # (emb @ ck1[:, :, t].T)[v, o], replicated across the 4 sequence bands
    W1f = np.zeros((128, 512), F32)
    for t in range(4):
        fold = emb_w @ ck1[:, :, t].T          # [26, 32]
        for s in range(4):
            W1f[32 * s:32 * s + 26, 128 * t + 32 * s:128 * t + 32 * s + 32] = fold
    cb1rep = np.tile(cb1, 4)[:, None]
    ck2q = np.concatenate([np.tile(ck2[:, :, t].T, (4, 1)) for t in range(6)], axis=1)
    ck3d = np.concatenate([np.tile(ck3[:, :, t].T, (2, 1)) for t in range(8)], axis=1)
    fw1a = np.ascontiguousarray(fw1[:128])
    fw1b = np.ascontiguousarray(fw1[128:224])
    fb1c = np.ascontiguousarray(fb1.reshape(4, 128).T)
    fw2p = np.ascontiguousarray(fw2.reshape(4, 128, 256).transpose(1, 0, 2).reshape(128, 1024))
    fb2c = np.ascontiguousarray(fb2.reshape(2, 128).T)
    fw3p = np.ascontiguousarray(fw3.reshape(2, 128).T)

    # ---------------- device program ----------------
    dt = mybir.dt
    AF = mybir.ActivationFunctionType
    OP = mybir.AluOpType
    rg = [list(range(NCORES))]
    bf = dt.bfloat16
    fr = dt.float32r
    PROT_BF = bool(int(os.environ.get("DTA_PROT_BF16", "0")))
    pd = bf if PROT_BF else fr

    def pmemset(ap):
        nc.vector.memset(ap if PROT_BF else ap.bitcast(dt.float32), 0.0)

    nc = bacc.Bacc("TRN2", target_bir_lowering=False, debug=False, num_devices=NCORES)

    def din(name, shape, dty=dt.float32):
        return nc.dram_tensor(name, list(shape), dty, kind="ExternalInput")

    W1_d, W2_d, W3_d = din("W1", [5, 64], fr), din("W2", [64, 128], fr), din("W3", [128, 128], fr)
    chv_d = din("chv", [128, 6])
    ident_d = din("ident", [128, 128], bf)
    identf_d = din("identf", [64, 64])
    pidx_d = din("pidx", [128, 1])
    idx_d = din("idxg", [128, nchunk * 128], dt.int16)
    S_d = din("Sm", [128, nblk * 128], bf)
    G1_d = din("G1m", [128, nblk * 8], bf)
    P_d = din("Pm", [128, 4 * NT * 128], bf)
    xT8_d = din("xT8", [8, NSH], bf)
    d2b_d = din("d2b", [128, NSH], bf)
    cntinv_d = din("cntinv", [64, 1])
    ids_d = din("ids4", [4, (BSH // 4) * SEQ_LEN], bf)
    sel_d = din("sel4", [4, 4 * VOCAB], bf)
    W1f_d = din("W1f", [128, 512], pd)
    ck2_d, ck3_d = din("ck2q", [128, 384], pd), din("ck3d", [128, 768], pd)
    cb1_d, cb2_d, cb3_d = din("cb1r", [128, 1]), din("cb2", [64, 1]), din("cb3", [96, 1])
    f1a_d, f1b_d = din("fw1a", [128, 512], pd), din("fw1b", [96, 512], pd)
    fb1_d = din("fb1c", [128, 4])
    f2_d, fb2_d = din("fw2p", [128, 1024], pd), din("fb2c", [128, 2])
    f3_d, fb3_d = din("fw3p", [128, 2], pd), din("fb3", [1, 1])
    out_d = nc.dram_tensor("out", [1, BSH], dt.float32, kind="ExternalOutput")

    with tile.TileContext(nc) as tc:
        with (
            tc.tile_pool(name="dram", bufs=1, space="DRAM") as dram,
            tc.tile_pool(name="cst", bufs=1) as cst,
            tc.tile_pool(name="big", bufs=1) as big,
            tc.tile_pool(name="wk", bufs=2) as wk,
            tc.tile_pool(name="wk1", bufs=1) as wk1,
            tc.tile_pool(name="gp", bufs=2, space="PSUM") as gp,
            tc.tile_pool(name="pp", bufs=4, space="PSUM") as pp,
        ):
            nc.gpsimd.load_library(mlp_lib)

            ag1_in = dram.tile([NSH, 128], bf)
            h1_full = dram.tile([N_NODES, 128], bf, addr_space="Shared")
            ag2_in = dram.tile([NSH, 128], bf)
            h2_full = dram.tile([N_NODES, 128], bf, addr_space="Shared")
            ar_ins = [dram.tile([128, 2], dt.float32, name=f"arin{i}") for i in range(3)]
            ar_outs = [dram.tile([128, 2], dt.float32, addr_space="Shared",
                                 name=f"arout{i}") for i in range(3)]
            arp_in = dram.tile([BATCH, 128], dt.float32)
            rsp_out = dram.tile([BSH, 128], dt.float32)

            def load(name, d, shape, dty=dt.float32):
                t = cst.tile(shape, dty, name=name)
                nc.sync.dma_start(t[:], d[:])
                return t

            ident_t = load("ident_t", ident_d, [128, 128], bf)
            identf_t = load("identf_t", identf_d, [64, 64])
            pidx_t = load("pidx_t", pidx_d, [128, 1])
            chv_t = load("chv_t", chv_d, [128, 6])
            W1_t = load("W1_t", W1_d, [5, 64], fr)
            W2_t = load("W2_t", W2_d, [64, 128], fr)
            W3_t = load("W3_t", W3_d, [128, 128], fr)
            idx_t = load("idx_t", idx_d, [128, nchunk * 128], dt.int16)
            cntinv_t = load("cntinv_t", cntinv_d, [64, 1])
            sel_t = load("sel_t", sel_d, [4, 4 * VOCAB], bf)
            W1f_t = load("W1f_t", W1f_d, [128, 512], pd)
            ck2_t = load("ck2_t", ck2_d, [128, 384], pd)
            ck3_t = load("ck3_t", ck3_d, [128, 768], pd)
            cb1_t = load("cb1_t", cb1_d, [128, 1])
            cb2_t = load("cb2_t", cb2_d, [64, 1])
            cb3_t = load("cb3_t", cb3_d, [96, 1])
            f1a_t = load("f1a_t", f1a_d, [128, 512], pd)
            f1b_t = load("f1b_t", f1b_d, [96, 512], pd)
            fb1_t = load("fb1_t", fb1_d, [128, 4])
            f2_t = load("f2_t", f2_d, [128, 1024], pd)
            fb2_t = load("fb2_t", fb2_d, [128, 2])
            f3_t = load("f3_t", f3_d, [128, 2], pd)
            fb3_t = load("fb3_t", fb3_d, [1, 1])
            xT8_t = load("xT8_t", xT8_d, [8, NSH], bf)
            d2b_t = load("d2b_t", d2b_d, [128, NSH], bf)
            S_sb = load("S_sb", S_d, [128, nblk * 128], bf)
            G1_sb = load("G1_sb", G1_d, [128, nblk * 8], bf)
            P_sb = load("P_sb", P_d, [128, 4 * NT * 128], bf)

            z_sb = big.tile([128, NSH], dt.float32)
            hT_sb = big.tile([128, NSH], bf)
            selfd = big.tile([128, NSH], bf)
            hnat = big.tile([128, NSH], bf)
            protT = big.tile([96, BSH], pd)
            c1T = [big.tile([128, BSH], pd, name=f"c1T_{j}") for j in range(4)]
            c2T = [big.tile([128, BSH], pd, name=f"c2T_{j}") for j in range(2)]

            nc.vector.memset(hnat[:], 0.0)
            # layer-1 self term: x^T * dis^2
            nc.vector.tensor_tensor(selfd[:8, :], xT8_t[:], d2b_t[:8, :], OP.mult)

            # ---------------- protein group ----------------
            def protein_group(gi):
                idsg = wk.tile([4, SEQ_LEN], bf, tag="idsg", name=f"idsg{gi}")
                nc.sync.dma_start(idsg[:], ids_d[:, SEQ_LEN * gi:SEQ_LEN * (gi + 1)])
                # one-hot of the 4 sequences, banded [32s+v], cols 1..1000
                oh = wk.tile([128, 1004], pd, tag="oh", name=f"oh{gi}")
                pmemset(oh[:, 0:1])
                pmemset(oh[:, 1001:1004])
                for (l0, Lc) in ((0, 512), (512, 488)):
                    pb = pp.tile([128, 512], dt.float32, space="PSUM", tag="pp",
                                 name=f"pb{gi}_{l0}")
                    for si in range(4):
                        nc.tensor.matmul(
                            pb[32 * si:32 * si + 26, :Lc],
                            lhsT=sel_t[:, VOCAB * si:VOCAB * (si + 1)],
                            rhs=idsg[:, l0:l0 + Lc],
                            start=True, stop=True, tile_position=(0, 32 * si))
                    nc.vector.tensor_scalar(oh[:, 1 + l0:1 + l0 + Lc], pb[:, :Lc],
                                            pidx_t[:], None, OP.is_equal)
                h1Q = wk.tile([128, 1003], pd, tag="h1Q", name=f"h1Q{gi}")
                pmemset(h1Q[:, 0:2])
                pmemset(h1Q[:, 1001:1003])
                for (l0, Lc) in ((0, 512), (512, 488)):
                    c1p = pp.tile([128, 512], dt.float32, space="PSUM", tag="pp",
                                  name=f"c1p{gi}_{l0}")
                    for tap in range(4):
                        nc.tensor.matmul(
                            c1p[:, :Lc],
                            lhsT=W1f_t[:, 128 * tap:128 * (tap + 1)],
                            rhs=oh[:, l0 + tap:l0 + tap + Lc],
                            start=(tap == 0), stop=(tap == 3))
                    nc.scalar.activation(h1Q[:, 2 + l0:2 + l0 + Lc], c1p[:, :Lc],
                                         AF.Relu, bias=cb1_t[:])
                # col 1001 got a junk 1000th conv value from the even-width
                # chunk; conv2 needs it zero
                pmemset(h1Q[:, 1001:1003])
                h2D = []
                for p in range(2):
                    h2p = wk.tile([128, 1005], pd, tag=f"h2D{p}",
                                  name=f"h2D{gi}_{p}")
                    pmemset(h2p[:, 0:3])
                    pmemset(h2p[:, 1001:1005])
                    h2D.append(h2p)
                for (l0, Lc) in ((0, 512), (512, 486)):
                    c2ps = []
                    for si in range(4):
                        c2p = pp.tile([64, 512], dt.float32, space="PSUM", tag="pp",
                                      name=f"c2p{gi}_{l0}_{si}")
                        c2ps.append(c2p)
                    for tap in range(6):
                        for si in range(4):
                            nc.tensor.matmul(
                                c2ps[si][:, :Lc],
                                lhsT=ck2_t[32 * si:32 * (si + 1),
                                           64 * tap:64 * (tap + 1)],
                                rhs=h1Q[32 * si:32 * (si + 1), l0 + tap:l0 + tap + Lc],
                                start=(tap == 0), stop=(tap == 5),
                                tile_position=(32 * si, 0))
                    for si in range(4):
                        nc.scalar.activation(
                            h2D[si // 2][64 * (si % 2):64 * (si % 2 + 1),
                                         3 + l0:3 + l0 + Lc],
                            c2ps[si][:, :Lc], AF.Relu, bias=cb2_t[:])
                for p in range(2):
                    mx = [wk.tile([96, 1], dt.float32, tag=f"mx{j}",
                                  name=f"mx{gi}_{p}_{j}") for j in range(2)]
                    tmp = [wk.tile([96, 1], dt.float32, tag=f"tm{j}",
                                   name=f"tm{gi}_{p}_{j}") for j in range(2)]
                    for (l0, Lc) in ((0, 512), (512, 486)):
                        Lreal = 512 if l0 == 0 else 485
                        c3ps = []
                        for j in range(2):
                            c3p = pp.tile([96, 512], dt.float32, space="PSUM", tag="pp",
                                          name=f"c3p{gi}_{p}_{l0}_{j}")
                            c3ps.append(c3p)
                        for tap in range(8):
                            for j in range(2):
                                nc.tensor.matmul(
                                    c3ps[j][:, :Lc],
                                    lhsT=ck3_t[64 * j:64 * (j + 1),
                                               96 * tap:96 * (tap + 1)],
                                    rhs=h2D[p][64 * j:64 * (j + 1),
                                               l0 + tap:l0 + tap + Lc],
                                    start=(tap == 0), stop=(tap == 7),
                                    tile_position=(64 * j, 0))
                        for j in range(2):
                            dst = mx[j] if l0 == 0 else tmp[j]
                            nc.vector.tensor_reduce(dst[:], c3ps[j][:, :Lreal],
                                                    axis=mybir.AxisListType.X, op=OP.max)
                            if l0 != 0:
                                nc.vector.tensor_tensor(mx[j][:], mx[j][:], tmp[j][:],
                                                        OP.max)
                    for j in range(2):
                        s_idx = 4 * gi + 2 * p + j
                        nc.scalar.activation(protT[:, s_idx:s_idx + 1], mx[j][:],
                                             AF.Relu, bias=cb3_t[:])

            pending = list(range(16))
            slot = [0]

            def filler(period=8):
                slot[0] += 1
                if pending and slot[0] % period == 0:
                    protein_group(pending.pop(0))

            # ---------------- GCN layer ----------------
            def gcn_layer(L, fin, fout, src_dram, Wt, period):
                NG = NT // 4
                zs = cst.tile([128, NG], dt.float32, name=f"zs{L}")
                zq = cst.tile([128, NG], dt.float32, name=f"zq{L}")
                sq_scr = wk1.tile([128, 512], dt.float32, tag="sqs", name=f"sqs{L}")
                Gt = None
                for gidx in range(NG):
                    aggS4 = wk.tile([128, 512], fr, tag="aggS",
                                    name=f"aggS{L}_{gidx}")
                    for tt in range(4):
                        t = 4 * gidx + tt
                        aggT = gp.tile([128, 128], dt.float32, space="PSUM", tag="aggp",
                                       name=f"agg{L}_{t}")
                        for k in range(bpt):
                            b = t * bpt + k
                            if L == 0:
                                lhsT = G1_sb[:, 8 * b:8 * b + 5]
                            else:
                                ci, bb = divmod(b, 16)
                                if bb == 0:
                                    Gt = wk.tile([128, 16, 128], bf, tag="gch",
                                                 name=f"g{L}_{ci}")
                                    nc.gpsimd.dma_gather(
                                        Gt[:], src_dram[:],
                                        idx_t[:, 128 * ci:128 * (ci + 1)], 2048, 2048,
                                        128, single_packet=False)
                                lhsT = Gt[:, bb, :fin]
                            nc.tensor.matmul(aggT[:fin, :], lhsT=lhsT,
                                             rhs=S_sb[:, 128 * b:128 * (b + 1)],
                                             start=(k == 0), stop=(k == bpt - 1))
                        nc.vector.tensor_tensor(aggS4[:fin, 128 * tt:128 * (tt + 1)],
                                                aggT[:fin, :],
                                                selfd[:fin, 128 * t:128 * (t + 1)],
                                                OP.add)
                        filler(period)
                    zT4 = pp.tile([128, 512], dt.float32, space="PSUM", tag="pp",
                                   name=f"z{L}_{gidx}")
                    nc.tensor.matmul(zT4[:fout, :], lhsT=Wt[:fin, :fout],
                                     rhs=aggS4[:fin, :], start=True, stop=True)
                    nc.scalar.activation(z_sb[:fout, 512 * gidx:512 * (gidx + 1)],
                                         zT4[:fout, :], AF.Copy,
                                         accum_out=zs[:fout, gidx:gidx + 1])
                    nc.scalar.activation(sq_scr[:fout, :], zT4[:fout, :], AF.Square,
                                         accum_out=zq[:fout, gidx:gidx + 1])
                ssum = wk.tile([128, 2], dt.float32, tag="ssum", name=f"ssum{L}")
                nc.vector.memset(ssum[:], 0.0)
                nc.vector.tensor_reduce(ssum[:fout, 0:1], zs[:fout, :],
                                        axis=mybir.AxisListType.X, op=OP.add)
                nc.vector.tensor_reduce(ssum[:fout, 1:2], zq[:fout, :],
                                        axis=mybir.AxisListType.X, op=OP.add)
                nc.sync.dma_start(ar_ins[L][:], ssum[:])
                nc.gpsimd.collective_compute(
                    "AllReduce", OP.add, replica_groups=rg,
                    ins=[ar_ins[L].opt()], outs=[ar_outs[L].opt()])
                stg = wk.tile([128, 2], dt.float32, tag="stg", name=f"stg{L}")
                nc.sync.dma_start(stg[:], ar_outs[L][:])
                vg = chv_t[:fout, 2 * L:2 * L + 1]
                vbt = chv_t[:fout, 2 * L + 1:2 * L + 2]
                mean = wk.tile([128, 1], dt.float32, tag="bnv0", name=f"mean{L}")
                ex2 = wk.tile([128, 1], dt.float32, tag="bnv1", name=f"ex2{L}")
                var = wk.tile([128, 1], dt.float32, tag="bnv2", name=f"var{L}")
                sd = wk.tile([128, 1], dt.float32, tag="bnv3", name=f"sd{L}")
                s_ch = wk.tile([128, 1], dt.float32, tag="bnv4", name=f"sch{L}")
                b_ch = wk.tile([128, 1], dt.float32, tag="bnv5", name=f"bch{L}")
                t1 = wk.tile([128, 1], dt.float32, tag="bnv6", name=f"t1{L}")
                nc.vector.tensor_scalar(mean[:fout], stg[:fout, 0:1], 1.0 / N_NODES,
                                        None, OP.mult)
                nc.vector.tensor_scalar(ex2[:fout], stg[:fout, 1:2], 1.0 / N_NODES,
                                        None, OP.mult)
                nc.vector.tensor_tensor(var[:fout], mean[:fout], mean[:fout], OP.mult)
                nc.vector.tensor_tensor(var[:fout], ex2[:fout], var[:fout], OP.subtract)
                nc.vector.tensor_scalar(var[:fout], var[:fout], 1e-5, None, OP.add)
                nc.scalar.activation(sd[:fout], var[:fout], AF.Sqrt)
                nc.vector.reciprocal(s_ch[:fout], sd[:fout])
                nc.vector.tensor_tensor(s_ch[:fout], s_ch[:fout], vg, OP.mult)
                nc.vector.tensor_tensor(t1[:fout], mean[:fout], s_ch[:fout], OP.mult)
                nc.vector.tensor_tensor(b_ch[:fout], vbt, t1[:fout], OP.subtract)
                return s_ch, b_ch

            def apply_bn(L, fout, s_ch, b_ch):
                for gidx in range(NT // 4):
                    nc.scalar.activation(
                        hT_sb[:fout, 512 * gidx:512 * (gidx + 1)],
                        z_sb[:fout, 512 * gidx:512 * (gidx + 1)],
                        AF.Relu, bias=b_ch[:fout], scale=s_ch[:fout])
                for t in range(NT):
                    tp = gp.tile([128, 128], bf, space="PSUM", tag="aggp",
                                 name=f"tp{L}_{t}")
                    nc.tensor.transpose(tp[:, :fout], hT_sb[:fout, 128 * t:128 * (t + 1)],
                                        ident_t[:fout, :fout])
                    nc.vector.tensor_copy(hnat[:, 128 * t:128 * t + fout],
                                          tp[:, :fout])
                if L < 2:
                    nc.vector.tensor_tensor(selfd[:fout, :], hT_sb[:fout, :],
                                            d2b_t[:fout, :], OP.mult)

            def _emit_regressor(drugT):
                for jc in range(4):
                    f1ps = pp.tile([128, 64], dt.float32, space="PSUM", tag="pp",
                                   name=f"f1ps{jc}")
                    nc.tensor.matmul(f1ps[:], lhsT=f1a_t[:, 128 * jc:128 * (jc + 1)],
                                     rhs=drugT[:], start=True, stop=False)
                    nc.tensor.matmul(f1ps[:], lhsT=f1b_t[:, 128 * jc:128 * (jc + 1)],
                                     rhs=protT[:], start=False, stop=True)
                    nc.scalar.activation(c1T[jc][:, :], f1ps[:], AF.Relu,
                                         bias=fb1_t[:, jc:jc + 1])
                for jc in range(2):
                    f2ps = pp.tile([128, 64], dt.float32, space="PSUM", tag="pp",
                                   name=f"f2ps{jc}")
                    for ic in range(4):
                        nc.tensor.matmul(
                            f2ps[:],
                            lhsT=f2_t[:, 256 * ic + 128 * jc:256 * ic + 128 * jc + 128],
                            rhs=c1T[ic][:, :], start=(ic == 0), stop=(ic == 3))
                    nc.scalar.activation(c2T[jc][:, :], f2ps[:], AF.Relu,
                                         bias=fb2_t[:, jc:jc + 1])
                f3ps = pp.tile([1, 64], dt.float32, space="PSUM", tag="pp", name="f3ps0")
                for ic in range(2):
                    nc.tensor.matmul(f3ps[:], lhsT=f3_t[:, ic:ic + 1],
                                     rhs=c2T[ic][:, :],
                                     start=(ic == 0), stop=(ic == 1))
                outs = wk.tile([1, 64], dt.float32, tag="outs", name="outs0")
                nc.vector.tensor_scalar(outs[:], f3ps[:], fb3_t[:1, 0:1], None, OP.add)
                nc.sync.dma_start(out_d[:], outs[:])

            # ================= emission =================
            s1, bb1 = gcn_layer(0, 5, 64, None, W1_t, 99)
            filler(1)
            apply_bn(0, 64, s1, bb1)
            view1 = ag1_in[:, :].rearrange("(t p) j -> p t j", p=128)
            nc.sync.dma_start(view1,
                              hnat[:, :].rearrange("p (t j) -> p t j", j=128))
            nc.gpsimd.collective_compute("AllGather", OP.bypass, replica_groups=rg,
                                         ins=[ag1_in.opt()], outs=[h1_full.opt()])
            filler(1)
            filler(1)

            s2c, bb2 = gcn_layer(1, 64, 128, h1_full, W2_t, 5)
            filler(1)
            apply_bn(1, 128, s2c, bb2)
            view2 = ag2_in[:, :].rearrange("(t p) j -> p t j", p=128)
            nc.sync.dma_start(view2,
                              hnat[:, :].rearrange("p (t j) -> p t j", j=128))
            nc.gpsimd.collective_compute("AllGather", OP.bypass, replica_groups=rg,
                                         ins=[ag2_in.opt()], outs=[h2_full.opt()])
            filler(1)
            filler(1)

            s3c, bb3 = gcn_layer(2, 128, 128, h2_full, W3_t, 7)
            filler(1)
            apply_bn(2, 128, s3c, bb3)

            # ---------------- pooling ----------------
            poolps = pp.tile([128, 512], dt.float32, space="PSUM", tag="pp",
                             name="poolps")
            for t in range(NT):
                nc.tensor.matmul(
                    poolps[:], lhsT=hnat[:, 128 * t:128 * (t + 1)],
                    rhs=P_sb[:, 512 * t:512 * (t + 1)],
                    start=(t == 0), stop=(t == NT - 1))
            poolS = wk1.tile([128, 512], bf, tag="poolS", name="poolS0")
            nc.vector.tensor_copy(poolS[:], poolps[:])
            for w in range(4):
                tpw = gp.tile([128, 128], bf, space="PSUM", tag="aggp", name=f"tpw{w}")
                nc.tensor.transpose(tpw[:], poolS[:, 128 * w:128 * (w + 1)], ident_t[:])
                parts = wk.tile([128, 128], dt.float32, tag="parts", name=f"parts{w}")
                nc.vector.tensor_copy(parts[:], tpw[:])
                nc.sync.dma_start(arp_in[128 * w:128 * (w + 1), :], parts[:])
            nc.gpsimd.collective_compute("ReduceScatter", OP.add, replica_groups=rg,
                                         ins=[arp_in.opt()], outs=[rsp_out.opt()])
            while pending:
                protein_group(pending.pop(0))
            drugsum = wk.tile([64, 128], dt.float32, tag="drugsum", name="drugsum0")
            nc.sync.dma_start(drugsum[:], rsp_out[:])
            drug = wk.tile([64, 128], dt.float32, tag="drug", name="drug0")
            nc.vector.tensor_scalar(drug[:], drugsum[:], cntinv_t[:], None, OP.mult)
            tpd = gp.tile([128, 128], dt.float32, space="PSUM", tag="aggp", name="tpd0")
            nc.tensor.transpose(tpd[:, :64], drug[:], identf_t[:])
            drugT = wk.tile([128, 64], pd, tag="drugT", name="drugT0")
            nc.vector.tensor_copy(drugT[:], tpd[:, :64])

            _emit_regressor(drugT)

    nc.compile()

    _pc = (lambda a: _bf(a)) if PROT_BF else (lambda a: a)
    in_maps = []
    for c in range(NCORES):
        in_maps.append({
            "W1": W1, "W2": W2, "W3": W3, "chv": chvec,
            "ident": _bf(ident), "identf": ident[:64, :64].copy(),
            "pidx": pidx128,
            "idxg": idx_w[c], "Sm": _bf(S_host[c]), "G1m": _bf(G1_host[c]),
            "Pm": _bf(P_host[c]),
            "xT8": _bf(xT8[c]), "d2b": _bf(d2b[c]),
            "cntinv": cntinv[64 * c:64 * (c + 1)][:, None],
            "ids4": _bf(ids4[c]), "sel4": _bf(sel4h),
            "W1f": _pc(W1f), "ck2q": _pc(ck2q), "ck3d": _pc(ck3d),
            "cb1r": cb1rep, "cb2": cb2[:, None], "cb3": cb3[:, None],
            "fw1a": _pc(fw1a), "fw1b": _pc(fw1b), "fb1c": fb1c,
            "fw2p": _pc(fw2p), "fb2c": fb2c, "fw3p": _pc(fw3p),
            "fb3": np.array([[fb3[0]]], F32),
        })

    res = run_bass_kernel_spmd(nc, in_maps, core_ids=list(range(NCORES)))
    LAST_RES = res
    out = np.concatenate([res.results[c]["out"][0] for c in range(NCORES)])
    return out.astype(F32)


# revision 20
# speedup vs baseline: 1.8403x; 1.0057x over previous
"""DTAModel (drug-target affinity) Trainium2 kernel — 8-core SPMD, single launch.

Sharding: node shards of 4096 (GCN path, dst-sharded edge aggregation with
AllGather of h between layers) + pair shards of 64 (protein CNN + regressor).
All float compute on device; host only builds index/coefficient tables.

v3: bf16 data path end-to-end (1 cyc/col matmuls + fast weight load), self
loops removed from the edge list (device self-term via hT*dis^2), layer-1
gather fully host-prepacked and SBUF-resident, S/P matrices SBUF-resident,
embedding folded into conv1 (rank-26), pool restructured with hnat stationary.
"""
import os
import sys
import types

import numpy as np

N_NODES = 32768
N_EDGES = 131072
BATCH = 512
SEQ_LEN = 1000
VOCAB = 26
EMB = 128
NCORES = 8
NSH = N_NODES // NCORES      # 4096 nodes per core
BSH = BATCH // NCORES        # 64 pairs per core
NT = NSH // 128              # 32 node tiles per core

F32 = np.float32
BF16 = np.dtype("bfloat16") if hasattr(np, "bfloat16") else None
LAST_RES = None


def _bf(x):
    import ml_dtypes
    return np.asarray(x, dtype=ml_dtypes.bfloat16)


def _install_ntff_shim():
    if "antenv.axon_hooks" in sys.modules:
        return
    mod = types.ModuleType("antenv.axon_hooks")
    holder = {"h": None}
    mod.set_axon_ntff_profile_hook = lambda h: holder.__setitem__("h", h)
    mod.get_axon_ntff_profile_hook = lambda: holder["h"]
    sys.modules["antenv.axon_hooks"] = mod
    try:
        from trn_agent_boot.trn_boot import _ntff_profile_via_ctypes
        so = "/opt/axon/libaxon_pjrt.so"
        if os.path.exists(so):
            mod.set_axon_ntff_profile_hook(_ntff_profile_via_ctypes(so))
    except Exception:
        pass


def _prep_edges(edge_index, x):
    """Edge tables without self-loops: per-core dst-sorted 128-edge blocks,
    dense S matrices (bf16), gather idx (int16), and dense layer-1 gathered
    x blocks (pure indexing of the input, done on host)."""
    src = np.asarray(edge_index[0], np.int64)
    dst = np.asarray(edge_index[1], np.int64)
    deg = (1.0 + np.bincount(dst, minlength=N_NODES)).astype(np.float64)
    dis = 1.0 / np.sqrt(deg)
    coef_all = (dis[src] * dis[dst]).astype(F32)
    order = np.argsort(dst, kind="stable")
    s_s, d_s, c_s = src[order], dst[order], coef_all[order]

    tile_of = d_s // 128
    counts = np.bincount(tile_of, minlength=N_NODES // 128)
    bpt = int(np.ceil(counts.max() / 128))
    nblk = NT * bpt

    idx = np.zeros((NCORES, nblk * 128), np.int64)
    dstl = np.zeros((NCORES, nblk * 128), np.int64)
    coef = np.zeros((NCORES, nblk * 128), F32)
    tstart = np.concatenate([[0], np.cumsum(counts)])
    for gt in range(N_NODES // 128):
        c, t = divmod(gt, NT)
        lo, hi = tstart[gt], tstart[gt + 1]
        n = hi - lo
        base = t * bpt * 128
        idx[c, base:base + n] = s_s[lo:hi]
        dstl[c, base:base + n] = d_s[lo:hi] - gt * 128
        coef[c, base:base + n] = c_s[lo:hi]

    # dense S: S_all[c][e, 128*b + j] = coef * (dstl == j)
    S_all = np.zeros((NCORES, nblk, 128, 128), F32)
    bix = np.tile(np.arange(nblk)[:, None], (1, 128)).ravel()
    eix = np.tile(np.arange(128)[None, :], (nblk, 1)).ravel()
    for c in range(NCORES):
        S_all[c][bix, eix, dstl[c]] = coef[c]
    S_all = S_all.transpose(0, 2, 1, 3).reshape(NCORES, 128, nblk * 128)

    nchunk = (nblk * 128 + 2047) // 2048
    idx16 = idx.astype(np.int16)
    wrapped = np.zeros((NCORES, 128, nchunk * 128), np.int16)
    for c in range(NCORES):
        w = idx16[c].reshape(nchunk, 128, 16)
        for ci in range(nchunk):
            blockw = w[ci].reshape(-1, 16).T
            wrapped[c, :, 128 * ci:128 * (ci + 1)] = np.tile(blockw, (8, 1))

    # layer-1 gathered x, dense block layout [128, nblk*8] (pads gather row 0,
    # killed by coef 0 in S)
    xpad8 = np.zeros((N_NODES, 8), F32)
    xpad8[:, :5] = x
    G1 = np.zeros((NCORES, 128, nblk * 8), F32)
    for c in range(NCORES):
        G1[c] = xpad8[idx[c]].reshape(nblk, 128, 8).transpose(1, 0, 2) \
            .reshape(128, nblk * 8)

    dis2 = (dis * dis).astype(F32)
    return bpt, nblk, nchunk, wrapped, S_all, G1, dis2


def kernel(**inputs):
    global LAST_RES
    _install_ntff_shim()
    import concourse.bacc as bacc
    import concourse.tile as tile
    from concourse import hw_specs
    # Tile's static schedule uses this cost model; the default badly
    # underestimates SWDGE gather descriptor generation (~8 ns/desc measured),
    # which starves the PE stream of filler work during gather windows.
    hw_specs.TRN2Spec.SWDGE_NS_PER_DESCRIPTOR = 8.0
    from concourse import mybir
    from concourse.bass_utils import run_bass_kernel_spmd
    from concourse.library_config import mlp as mlp_lib

    g = lambda k: np.ascontiguousarray(np.asarray(inputs[k], F32))
    x = g("x")
    batch = np.asarray(inputs["batch"], np.int64)
    seq = np.asarray(inputs["protein_seq"], np.int64)
    W1 = g("W1")
    W2 = g("W2")
    W3 = g("W3")
    g1, bt1, g2, bt2, g3, bt3 = g("g1"), g("bt1"), g("g2"), g("bt2"), g("g3"), g("bt3")
    emb_w = g("emb")
    ck1, cb1, ck2, cb2, ck3, cb3 = g("ck1"), g("cb1"), g("ck2"), g("cb2"), g("ck3"), g("cb3")
    fw1, fb1, fw2, fb2, fw3, fb3 = g("fw1"), g("fb1"), g("fw2"), g("fb2"), g("fw3"), g("fb3")

    # ---------------- host tables ----------------
    bpt, nblk, nchunk, idx_w, S_host, G1_host, dis2 = _prep_edges(
        np.asarray(inputs["edge_index"]), x)

    cnt = np.bincount(batch, minlength=BATCH).astype(F32)
    cntinv = (1.0 / np.maximum(cnt, 1.0)).astype(F32)
    # pooling one-hots, t-major: P_all[c][n, t*512 + batch[node]] (batch < 512)
    P_host = np.zeros((NCORES, 128, NT * 512), F32)
    for c in range(NCORES):
        bl = batch[NSH * c:NSH * (c + 1)].reshape(NT, 128)
        for t in range(NT):
            P_host[c, np.arange(128), t * 512 + bl[t]] = 1.0

    chvec = np.zeros((128, 6), F32)
    chvec[:64, 0], chvec[:64, 1] = g1, bt1
    chvec[:, 2], chvec[:, 3] = g2, bt2
    chvec[:, 4], chvec[:, 5] = g3, bt3

    ident = np.eye(128, dtype=F32)
    # per-core self-term inputs: x^T (padded to 8 rows) and dis^2 broadcast
    xT8 = np.zeros((NCORES, 8, NSH), F32)
    d2b = np.zeros((NCORES, 128, NSH), F32)
    for c in range(NCORES):
        xT8[c, :5, :] = x[NSH * c:NSH * (c + 1)].T
        d2b[c, :, :] = dis2[NSH * c:NSH * (c + 1)][None, :]

    ids4 = seq.astype(F32).reshape(NCORES, BSH // 4, 4, SEQ_LEN).transpose(0, 2, 1, 3) \
        .reshape(NCORES, 4, (BSH // 4) * SEQ_LEN).copy()
    sel4h = np.zeros((4, 4 * VOCAB), F32)
    for s in range(4):
        sel4h[s, VOCAB * s:VOCAB * (s + 1)] = 1.0
    # pidx bands: partition 32*s + v compares against v (pad rows never read)
    pidx128 = np.zeros((128, 1), F32)
    for s in range(4):
        pidx128[32 * s:32 * s + 32, 0] = np.arange(32)
    # conv1 with embedding folded in (rank 26): W1f[32*s+v, 32*t+o] =
    # (emb @ ck1[:, :, t].T)[v, o], replicated across the 4 sequence bands
    W1f = np.zeros((128, 512), F32)
    for t in range(4):
        fold = emb_w @ ck1[:, :, t].T          # [26, 32]
        for s in range(4):
            W1f[32 * s:32 * s + 26, 128 * t + 32 * s:128 * t + 32 * s + 32] = fold
    cb1rep = np.tile(cb1, 4)[:, None]
    ck2q = np.concatenate([np.tile(ck2[:, :, t].T, (4, 1)) for t in range(6)], axis=1)
    ck3d = np.concatenate([np.tile(ck3[:, :, t].T, (2, 1)) for t in range(8)], axis=1)
    fw1a = np.ascontiguousarray(fw1[:128])
    fw1b = np.ascontiguousarray(fw1[128:224])
    fb1c = np.ascontiguousarray(fb1.reshape(4, 128).T)
    fw2p = np.ascontiguousarray(fw2.reshape(4, 128, 256).transpose(1, 0, 2).reshape(128, 1024))
    fb2c = np.ascontiguousarray(fb2.reshape(2, 128).T)
    fw3p = np.ascontiguousarray(fw3.reshape(2, 128).T)

    # ---------------- device program ----------------
    dt = mybir.dt
    AF = mybir.ActivationFunctionType
    OP = mybir.AluOpType
    rg = [list(range(NCORES))]
    bf = dt.bfloat16
    fr = dt.float32r
    mask = int(os.environ.get("DTA_FR_MASK", "15"))
    if int(os.environ.get("DTA_PROT_BF16", "0")):
        mask = 0
    dt1 = fr if mask & 1 else bf      # conv1: W1f, oh
    dt2 = fr if mask & 2 else bf      # conv2: ck2q, h1Q
    dt3 = fr if mask & 4 else bf      # conv3: ck3d, h2D
    dt4 = fr if mask & 8 else bf      # regressor: f*, c1T/c2T/protT/drugT

    def pmemset(ap):
        nc.vector.memset(ap if ap.dtype == bf else ap.bitcast(dt.float32), 0.0)

    nc = bacc.Bacc("TRN2", target_bir_lowering=False, debug=False, num_devices=NCORES)

    def din(name, shape, dty=dt.float32):
        return nc.dram_tensor(name, list(shape), dty, kind="ExternalInput")

    W1_d, W2_d, W3_d = din("W1", [5, 64], fr), din("W2", [64, 128], fr), din("W3", [128, 128], fr)
    chv_d = din("chv", [128, 6])
    ident_d = din("ident", [128, 128], bf)
    identf_d = din("identf", [64, 64])
    pidx_d = din("pidx", [128, 1])
    idx_d = din("idxg", [128, nchunk * 128], dt.int16)
    S_d = din("Sm", [128, nblk * 128], bf)
    G1_d = din("G1m", [128, nblk * 8], bf)
    P_d = din("Pm", [128, 4 * NT * 128], bf)
    xT8_d = din("xT8", [8, NSH], bf)
    d2b_d = din("d2b", [128, NSH], bf)
    cntinv_d = din("cntinv", [64, 1])
    ids_d = din("ids4", [4, (BSH // 4) * SEQ_LEN], bf)
    sel_d = din("sel4", [4, 4 * VOCAB], bf)
    W1f_d = din("W1f", [128, 512], dt1)
    ck2_d, ck3_d = din("ck2q", [128, 384], dt2), din("ck3d", [128, 768], dt3)
    cb1_d, cb2_d, cb3_d = din("cb1r", [128, 1]), din("cb2", [64, 1]), din("cb3", [96, 1])
    f1a_d, f1b_d = din("fw1a", [128, 512], dt4), din("fw1b", [96, 512], dt4)
    fb1_d = din("fb1c", [128, 4])
    f2_d, fb2_d = din("fw2p", [128, 1024], dt4), din("fb2c", [128, 2])
    f3_d, fb3_d = din("fw3p", [128, 2], dt4), din("fb3", [1, 1])
    out_d = nc.dram_tensor("out", [1, BSH], dt.float32, kind="ExternalOutput")
    DBG = bool(int(os.environ.get("DTA_DEBUG", "0")))
    dbg_d = nc.dram_tensor("dbg", [224, BSH], dt.float32, kind="ExternalOutput") if DBG else None

    with tile.TileContext(nc) as tc:
        with (
            tc.tile_pool(name="dram", bufs=1, space="DRAM") as dram,
            tc.tile_pool(name="cst", bufs=1) as cst,
            tc.tile_pool(name="big", bufs=1) as big,
            tc.tile_pool(name="wk", bufs=2) as wk,
            tc.tile_pool(name="wk1", bufs=1) as wk1,
            tc.tile_pool(name="gp", bufs=2, space="PSUM") as gp,
            tc.tile_pool(name="pp", bufs=4, space="PSUM") as pp,
        ):
            nc.gpsimd.load_library(mlp_lib)

            ag1_in = dram.tile([NSH, 128], bf)
            h1_full = dram.tile([N_NODES, 128], bf, addr_space="Shared")
            ag2_in = dram.tile([NSH, 128], bf)
            h2_full = dram.tile([N_NODES, 128], bf, addr_space="Shared")
            ar_ins = [dram.tile([128, 2], dt.float32, name=f"arin{i}") for i in range(3)]
            ar_outs = [dram.tile([128, 2], dt.float32, addr_space="Shared",
                                 name=f"arout{i}") for i in range(3)]
            arp_in = dram.tile([BATCH, 128], dt.float32)
            rsp_out = dram.tile([BSH, 128], dt.float32)

            def load(name, d, shape, dty=dt.float32):
                t = cst.tile(shape, dty, name=name)
                nc.sync.dma_start(t[:], d[:])
                return t

            ident_t = load("ident_t", ident_d, [128, 128], bf)
            identf_t = load("identf_t", identf_d, [64, 64])
            pidx_t = load("pidx_t", pidx_d, [128, 1])
            chv_t = load("chv_t", chv_d, [128, 6])
            W1_t = load("W1_t", W1_d, [5, 64], fr)
            W2_t = load("W2_t", W2_d, [64, 128], fr)
            W3_t = load("W3_t", W3_d, [128, 128], fr)
            idx_t = load("idx_t", idx_d, [128, nchunk * 128], dt.int16)
            cntinv_t = load("cntinv_t", cntinv_d, [64, 1])
            sel_t = load("sel_t", sel_d, [4, 4 * VOCAB], bf)
            W1f_t = load("W1f_t", W1f_d, [128, 512], dt1)
            ck2_t = load("ck2_t", ck2_d, [128, 384], dt2)
            ck3_t = load("ck3_t", ck3_d, [128, 768], dt3)
            cb1_t = load("cb1_t", cb1_d, [128, 1])
            cb2_t = load("cb2_t", cb2_d, [64, 1])
            cb3_t = load("cb3_t", cb3_d, [96, 1])
            f1a_t = load("f1a_t", f1a_d, [128, 512], dt4)
            f1b_t = load("f1b_t", f1b_d, [96, 512], dt4)
            fb1_t = load("fb1_t", fb1_d, [128, 4])
            f2_t = load("f2_t", f2_d, [128, 1024], dt4)
            fb2_t = load("fb2_t", fb2_d, [128, 2])
            f3_t = load("f3_t", f3_d, [128, 2], dt4)
            fb3_t = load("fb3_t", fb3_d, [1, 1])
            xT8_t = load("xT8_t", xT8_d, [8, NSH], bf)
            d2b_t = load("d2b_t", d2b_d, [128, NSH], bf)
            S_sb = load("S_sb", S_d, [128, nblk * 128], bf)
            G1_sb = load("G1_sb", G1_d, [128, nblk * 8], bf)
            P_sb = load("P_sb", P_d, [128, 4 * NT * 128], bf)

            z_sb = big.tile([128, NSH], dt.float32)
            hT_sb = big.tile([128, NSH], bf)
            selfd = big.tile([128, NSH], bf)
            hnat = big.tile([128, NSH], bf)
            protT = big.tile([96, BSH], dt4)
            c1T = [big.tile([128, BSH], dt4, name=f"c1T_{j}") for j in range(4)]
            c2T = [big.tile([128, BSH], dt4, name=f"c2T_{j}") for j in range(2)]

            nc.vector.memset(hnat[:], 0.0)
            # layer-1 self term: x^T * dis^2
            nc.vector.tensor_tensor(selfd[:8, :], xT8_t[:], d2b_t[:8, :], OP.mult)

            # ---------------- protein group ----------------
            def protein_group(gi):
                idsg = wk.tile([4, SEQ_LEN], bf, tag="idsg", name=f"idsg{gi}")
                nc.sync.dma_start(idsg[:], ids_d[:, SEQ_LEN * gi:SEQ_LEN * (gi + 1)])
                # one-hot of the 4 sequences, banded [32s+v], cols 1..1000
                oh = wk.tile([128, 1004], dt1, tag="oh", name=f"oh{gi}")
                pmemset(oh[:, 0:1])
                pmemset(oh[:, 1001:1004])
                for (l0, Lc) in ((0, 512), (512, 488)):
                    pb = pp.tile([128, 512], dt.float32, space="PSUM", tag="pp",
                                 name=f"pb{gi}_{l0}")
                    for si in range(4):
                        nc.tensor.matmul(
                            pb[32 * si:32 * si + 26, :Lc],
                            lhsT=sel_t[:, VOCAB * si:VOCAB * (si + 1)],
                            rhs=idsg[:, l0:l0 + Lc],
                            start=True, stop=True, tile_position=(0, 32 * si))
                    nc.vector.tensor_scalar(oh[:, 1 + l0:1 + l0 + Lc], pb[:, :Lc],
                                            pidx_t[:], None, OP.is_equal)
                h1Q = wk.tile([128, 1003], dt2, tag="h1Q", name=f"h1Q{gi}")
                pmemset(h1Q[:, 0:2])
                pmemset(h1Q[:, 1001:1003])
                for (l0, Lc) in ((0, 512), (512, 488)):
                    c1p = pp.tile([128, 512], dt.float32, space="PSUM", tag="pp",
                                  name=f"c1p{gi}_{l0}")
                    for tap in range(4):
                        nc.tensor.matmul(
                            c1p[:, :Lc],
                            lhsT=W1f_t[:, 128 * tap:128 * (tap + 1)],
                            rhs=oh[:, l0 + tap:l0 + tap + Lc],
                            start=(tap == 0), stop=(tap == 3))
                    nc.scalar.activation(h1Q[:, 2 + l0:2 + l0 + Lc], c1p[:, :Lc],
                                         AF.Relu, bias=cb1_t[:])
                # col 1001 got a junk 1000th conv value from the even-width
                # chunk; conv2 needs it zero
                pmemset(h1Q[:, 1001:1003])
                h2D = []
                for p in range(2):
                    h2p = wk.tile([128, 1005], dt3, tag=f"h2D{p}",
                                  name=f"h2D{gi}_{p}")
                    pmemset(h2p[:, 0:3])
                    pmemset(h2p[:, 1001:1005])
                    h2D.append(h2p)
                for (l0, Lc) in ((0, 512), (512, 486)):
                    c2ps = []
                    for si in range(4):
                        c2p = pp.tile([64, 512], dt.float32, space="PSUM", tag="pp",
                                      name=f"c2p{gi}_{l0}_{si}")
                        c2ps.append(c2p)
                    for tap in range(6):
                        for si in range(4):
                            nc.tensor.matmul(
                                c2ps[si][:, :Lc],
                                lhsT=ck2_t[32 * si:32 * (si + 1),
                                           64 * tap:64 * (tap + 1)],
                                rhs=h1Q[32 * si:32 * (si + 1), l0 + tap:l0 + tap + Lc],
                                start=(tap == 0), stop=(tap == 5),
                                tile_position=(32 * si, 0))
                    for si in range(4):
                        nc.scalar.activation(
                            h2D[si // 2][64 * (si % 2):64 * (si % 2 + 1),
                                         3 + l0:3 + l0 + Lc],
                            c2ps[si][:, :Lc], AF.Relu, bias=cb2_t[:])
                for p in range(2):
                    mx = [wk.tile([96, 1], dt.float32, tag=f"mx{j}",
                                  name=f"mx{gi}_{p}_{j}") for j in range(2)]
                    tmp = [wk.tile([96, 1], dt.float32, tag=f"tm{j}",
                                   name=f"tm{gi}_{p}_{j}") for j in range(2)]
                    for (l0, Lc) in ((0, 512), (512, 486)):
                        Lreal = 512 if l0 == 0 else 485
                        c3ps = []
                        for j in range(2):
                            c3p = pp.tile([96, 512], dt.float32, space="PSUM", tag="pp",
                                          name=f"c3p{gi}_{p}_{l0}_{j}")
                            c3ps.append(c3p)
                        for tap in range(8):
                            for j in range(2):
                                nc.tensor.matmul(
                                    c3ps[j][:, :Lc],
                                    lhsT=ck3_t[64 * j:64 * (j + 1),
                                               96 * tap:96 * (tap + 1)],
                                    rhs=h2D[p][64 * j:64 * (j + 1),
                                               l0 + tap:l0 + tap + Lc],
                                    start=(tap == 0), stop=(tap == 7),
                                    tile_position=(64 * j, 0))
                        for j in range(2):
                            dst = mx[j] if l0 == 0 else tmp[j]
                            nc.vector.tensor_reduce(dst[:], c3ps[j][:, :Lreal],
                                                    axis=mybir.AxisListType.X, op=OP.max)
                            if l0 != 0:
                                nc.vector.tensor_tensor(mx[j][:], mx[j][:], tmp[j][:],
                                                        OP.max)
                    for j in range(2):
                        s_idx = 4 * gi + 2 * p + j
                        nc.scalar.activation(protT[:, s_idx:s_idx + 1], mx[j][:],
                                             AF.Relu, bias=cb3_t[:])

            pending = list(range(16))
            slot = [0]

            def filler(period=8):
                slot[0] += 1
                if pending and slot[0] % period == 0:
                    protein_group(pending.pop(0))

            # ---------------- GCN layer ----------------
            def gcn_layer(L, fin, fout, src_dram, Wt, period):
                NG = NT // 4
                zs = cst.tile([128, NG], dt.float32, name=f"zs{L}")
                zq = cst.tile([128, NG], dt.float32, name=f"zq{L}")
                sq_scr = wk1.tile([128, 512], dt.float32, tag="sqs", name=f"sqs{L}")
                Gt = None
                for gidx in range(NG):
                    aggS4 = wk.tile([128, 512], fr, tag="aggS",
                                    name=f"aggS{L}_{gidx}")
                    for tt in range(4):
                        t = 4 * gidx + tt
                        aggT = gp.tile([128, 128], dt.float32, space="PSUM", tag="aggp",
                                       name=f"agg{L}_{t}")
                        for k in range(bpt):
                            b = t * bpt + k
                            if L == 0:
                                lhsT = G1_sb[:, 8 * b:8 * b + 5]
                            else:
                                ci, bb = divmod(b, 16)
                                if bb == 0:
                                    Gt = wk.tile([128, 16, 128], bf, tag="gch",
                                                 name=f"g{L}_{ci}")
                                    nc.gpsimd.dma_gather(
                                        Gt[:], src_dram[:],
                                        idx_t[:, 128 * ci:128 * (ci + 1)], 2048, 2048,
                                        128, single_packet=False)
                                lhsT = Gt[:, bb, :fin]
                            nc.tensor.matmul(aggT[:fin, :], lhsT=lhsT,
                                             rhs=S_sb[:, 128 * b:128 * (b + 1)],
                                             start=(k == 0), stop=(k == bpt - 1))
                        nc.vector.tensor_tensor(aggS4[:fin, 128 * tt:128 * (tt + 1)],
                                                aggT[:fin, :],
                                                selfd[:fin, 128 * t:128 * (t + 1)],
                                                OP.add)
                        filler(period)
                    zT4 = pp.tile([128, 512], dt.float32, space="PSUM", tag="pp",
                                   name=f"z{L}_{gidx}")
                    nc.tensor.matmul(zT4[:fout, :], lhsT=Wt[:fin, :fout],
                                     rhs=aggS4[:fin, :], start=True, stop=True)
                    nc.scalar.activation(z_sb[:fout, 512 * gidx:512 * (gidx + 1)],
                                         zT4[:fout, :], AF.Copy,
                                         accum_out=zs[:fout, gidx:gidx + 1])
                    nc.scalar.activation(sq_scr[:fout, :], zT4[:fout, :], AF.Square,
                                         accum_out=zq[:fout, gidx:gidx + 1])
                ssum = wk.tile([128, 2], dt.float32, tag="ssum", name=f"ssum{L}")
                nc.vector.memset(ssum[:], 0.0)
                nc.vector.tensor_reduce(ssum[:fout, 0:1], zs[:fout, :],
                                        axis=mybir.AxisListType.X, op=OP.add)
                nc.vector.tensor_reduce(ssum[:fout, 1:2], zq[:fout, :],
                                        axis=mybir.AxisListType.X, op=OP.add)
                nc.sync.dma_start(ar_ins[L][:], ssum[:])
                nc.gpsimd.collective_compute(
                    "AllReduce", OP.add, replica_groups=rg,
                    ins=[ar_ins[L].opt()], outs=[ar_outs[L].opt()])
                stg = wk.tile([128, 2], dt.float32, tag="stg", name=f"stg{L}")
                nc.sync.dma_start(stg[:], ar_outs[L][:])
                vg = chv_t[:fout, 2 * L:2 * L + 1]
                vbt = chv_t[:fout, 2 * L + 1:2 * L + 2]
                mean = wk.tile([128, 1], dt.float32, tag="bnv0", name=f"mean{L}")
                ex2 = wk.tile([128, 1], dt.float32, tag="bnv1", name=f"ex2{L}")
                var = wk.tile([128, 1], dt.float32, tag="bnv2", name=f"var{L}")
                sd = wk.tile([128, 1], dt.float32, tag="bnv3", name=f"sd{L}")
                s_ch = wk.tile([128, 1], dt.float32, tag="bnv4", name=f"sch{L}")
                b_ch = wk.tile([128, 1], dt.float32, tag="bnv5", name=f"bch{L}")
                t1 = wk.tile([128, 1], dt.float32, tag="bnv6", name=f"t1{L}")
                nc.vector.tensor_scalar(mean[:fout], stg[:fout, 0:1], 1.0 / N_NODES,
                                        None, OP.mult)
                nc.vector.tensor_scalar(ex2[:fout], stg[:fout, 1:2], 1.0 / N_NODES,
                                        None, OP.mult)
                nc.vector.tensor_tensor(var[:fout], mean[:fout], mean[:fout], OP.mult)
                nc.vector.tensor_tensor(var[:fout], ex2[:fout], var[:fout], OP.subtract)
                nc.vector.tensor_scalar(var[:fout], var[:fout], 1e-5, None, OP.add)
                nc.scalar.activation(sd[:fout], var[:fout], AF.Sqrt)
                nc.vector.reciprocal(s_ch[:fout], sd[:fout])
                nc.vector.tensor_tensor(s_ch[:fout], s_ch[:fout], vg, OP.mult)
                nc.vector.tensor_tensor(t1[:fout], mean[:fout], s_ch[:fout], OP.mult)
                nc.vector.tensor_tensor(b_ch[:fout], vbt, t1[:fout], OP.subtract)
                return s_ch, b_ch

            def apply_bn(L, fout, s_ch, b_ch):
                for gidx in range(NT // 4):
                    nc.scalar.activation(
                        hT_sb[:fout, 512 * gidx:512 * (gidx + 1)],
                        z_sb[:fout, 512 * gidx:512 * (gidx + 1)],
                        AF.Relu, bias=b_ch[:fout], scale=s_ch[:fout])
                for t in range(NT):
                    tp = gp.tile([128, 128], bf, space="PSUM", tag="aggp",
                                 name=f"tp{L}_{t}")
                    nc.tensor.transpose(tp[:, :fout], hT_sb[:fout, 128 * t:128 * (t + 1)],
                                        ident_t[:fout, :fout])
                    nc.vector.tensor_copy(hnat[:, 128 * t:128 * t + fout],
                                          tp[:, :fout])
                if L < 2:
                    nc.vector.tensor_tensor(selfd[:fout, :], hT_sb[:fout, :],
                                            d2b_t[:fout, :], OP.mult)

            def _emit_regressor(drugT):
                for jc in range(4):
                    f1ps = pp.tile([128, 64], dt.float32, space="PSUM", tag="pp",
                                   name=f"f1ps{jc}")
                    nc.tensor.matmul(f1ps[:], lhsT=f1a_t[:, 128 * jc:128 * (jc + 1)],
                                     rhs=drugT[:], start=True, stop=False)
                    nc.tensor.matmul(f1ps[:], lhsT=f1b_t[:, 128 * jc:128 * (jc + 1)],
                                     rhs=protT[:], start=False, stop=True)
                    nc.scalar.activation(c1T[jc][:, :], f1ps[:], AF.Relu,
                                         bias=fb1_t[:, jc:jc + 1])
                for jc in range(2):
                    f2ps = pp.tile([128, 64], dt.float32, space="PSUM", tag="pp",
                                   name=f"f2ps{jc}")
                    for ic in range(4):
                        nc.tensor.matmul(
                            f2ps[:],
                            lhsT=f2_t[:, 256 * ic + 128 * jc:256 * ic + 128 * jc + 128],
                            rhs=c1T[ic][:, :], start=(ic == 0), stop=(ic == 3))
                    nc.scalar.activation(c2T[jc][:, :], f2ps[:], AF.Relu,
                                         bias=fb2_t[:, jc:jc + 1])
                f3ps = pp.tile([1, 64], dt.float32, space="PSUM", tag="pp", name="f3ps0")
                for ic in range(2):
                    nc.tensor.matmul(f3ps[:], lhsT=f3_t[:, ic:ic + 1],
                                     rhs=c2T[ic][:, :],
                                     start=(ic == 0), stop=(ic == 1))
                outs = wk.tile([1, 64], dt.float32, tag="outs", name="outs0")
                nc.vector.tensor_scalar(outs[:], f3ps[:], fb3_t[:1, 0:1], None, OP.add)
                nc.sync.dma_start(out_d[:], outs[:])
                if DBG:
                    dbgt = wk1.tile([224, 64], dt.float32, tag="dbgt", name="dbgt0")
                    nc.vector.tensor_copy(dbgt[:128, :], drugT[:])
                    nc.vector.tensor_copy(dbgt[128:224, :], protT[:])
                    nc.sync.dma_start(dbg_d[:], dbgt[:])

            # ================= emission =================
            s1, bb1 = gcn_layer(0, 5, 64, None, W1_t, 99)
            filler(1)
            apply_bn(0, 64, s1, bb1)
            view1 = ag1_in[:, :].rearrange("(t p) j -> p t j", p=128)
            nc.sync.dma_start(view1,
                              hnat[:, :].rearrange("p (t j) -> p t j", j=128))
            nc.gpsimd.collective_compute("AllGather", OP.bypass, replica_groups=rg,
                                         ins=[ag1_in.opt()], outs=[h1_full.opt()])
            filler(1)
            filler(1)

            s2c, bb2 = gcn_layer(1, 64, 128, h1_full, W2_t, 5)
            filler(1)
            apply_bn(1, 128, s2c, bb2)
            view2 = ag2_in[:, :].rearrange("(t p) j -> p t j", p=128)
            nc.sync.dma_start(view2,
                              hnat[:, :].rearrange("p (t j) -> p t j", j=128))
            nc.gpsimd.collective_compute("AllGather", OP.bypass, replica_groups=rg,
                                         ins=[ag2_in.opt()], outs=[h2_full.opt()])
            filler(1)
            filler(1)

            s3c, bb3 = gcn_layer(2, 128, 128, h2_full, W3_t, 7)
            filler(1)
            apply_bn(2, 128, s3c, bb3)

            # ---------------- pooling ----------------
            poolps = pp.tile([128, 512], dt.float32, space="PSUM", tag="pp",
                             name="poolps")
            for t in range(NT):
                nc.tensor.matmul(
                    poolps[:], lhsT=hnat[:, 128 * t:128 * (t + 1)],
                    rhs=P_sb[:, 512 * t:512 * (t + 1)],
                    start=(t == 0), stop=(t == NT - 1))
            poolS = wk1.tile([128, 512], bf, tag="poolS", name="poolS0")
            nc.vector.tensor_copy(poolS[:], poolps[:])
            for w in range(4):
                tpw = gp.tile([128, 128], bf, space="PSUM", tag="aggp", name=f"tpw{w}")
                nc.tensor.transpose(tpw[:], poolS[:, 128 * w:128 * (w + 1)], ident_t[:])
                parts = wk.tile([128, 128], dt.float32, tag="parts", name=f"parts{w}")
                nc.vector.tensor_copy(parts[:], tpw[:])
                nc.sync.dma_start(arp_in[128 * w:128 * (w + 1), :], parts[:])
            nc.gpsimd.collective_compute("ReduceScatter", OP.add, replica_groups=rg,
                                         ins=[arp_in.opt()], outs=[rsp_out.opt()])
            while pending:
                protein_group(pending.pop(0))
            drugsum = wk.tile([64, 128], dt.float32, tag="drugsum", name="drugsum0")
            nc.sync.dma_start(drugsum[:], rsp_out[:])
            drug = wk.tile([64, 128], dt.float32, tag="drug", name="drug0")
            nc.vector.tensor_scalar(drug[:], drugsum[:], cntinv_t[:], None, OP.mult)
            tpd = gp.tile([128, 128], dt.float32, space="PSUM", tag="aggp", name="tpd0")
            nc.tensor.transpose(tpd[:, :64], drug[:], identf_t[:])
            drugT = wk.tile([128, 64], dt4, tag="drugT", name="drugT0")
            nc.vector.tensor_copy(drugT[:], tpd[:, :64])

            _emit_regressor(drugT)

    nc.compile()

    _c1 = (lambda a: a) if mask & 1 else _bf
    _c2 = (lambda a: a) if mask & 2 else _bf
    _c3 = (lambda a: a) if mask & 4 else _bf
    _c4 = (lambda a: a) if mask & 8 else _bf
    in_maps = []
    for c in range(NCORES):
        in_maps.append({
            "W1": W1, "W2": W2, "W3": W3, "chv": chvec,
            "ident": _bf(ident), "identf": ident[:64, :64].copy(),
            "pidx": pidx128,
            "idxg": idx_w[c], "Sm": _bf(S_host[c]), "G1m": _bf(G1_host[c]),
            "Pm": _bf(P_host[c]),
            "xT8": _bf(xT8[c]), "d2b": _bf(d2b[c]),
            "cntinv": cntinv[64 * c:64 * (c + 1)][:, None],
            "ids4": _bf(ids4[c]), "sel4": _bf(sel4h),
            "W1f": _c1(W1f), "ck2q": _c2(ck2q), "ck3d": _c3(ck3d),
            "cb1r": cb1rep, "cb2": cb2[:, None], "cb3": cb3[:, None],
            "fw1a": _c4(fw1a), "fw1b": _c4(fw1b), "fb1c": fb1c,
            "fw2p": _c4(fw2p), "fb2c": fb2c, "fw3p": _c4(fw3p),
            "fb3": np.array([[fb3[0]]], F32),
        })

    res = run_bass_kernel_spmd(nc, in_maps, core_ids=list(range(NCORES)))
    LAST_RES = res
    out = np.concatenate([res.results[c]["out"][0] for c in range(NCORES)])
    return out.astype(F32)
